# revision 2
# baseline (speedup 1.0000x reference)
"""Trainium2 Bass kernel for the 5-head detection tower (nn_DFD_10849087390476).

Network (per head h of 5): 1x1-conv tower on x [B,64,H,W]:
    h1 = relu(Win x + bin)
    h2 = h1 + relu(Wh0 h1 + bh0)
    h3 = h2 + relu(Wh1 h2 + bh1)
    out_h = Wout h3 + bout
Output = concat over heads: channels [cls 81, obj 2, box 4, pos 64, ins 128] = 279.

Sharding: data-parallel over (batch, H/2) -> 8 shards of 32768 pixels.
Per core the pixels form two 16384-px groups (A, B); a pair-tile is 512 px of
each.  Heads are paired on partition halves: co=(cls top, obj bottom),
bp=(box top, pos bottom), ins=(A top, B bottom).

All GEMMs have K=64.  in-proj and ins-out are 64x128 row-tiled matmul pairs
(tile_position (0,0)/(64,0)) running concurrently on the two PE row halves;
hidden layers and co/bp out are 128x128 block-diagonal.

PSUM is organized as two rings shared by every stage so the PE can run ahead
of evacuation: a 2-bank [128,1024] ring (bufs=3) for co/bp/out tiles and a
1-bank [128,512] ring (bufs=2) for ins tiles -- all 8 banks.  Evacuation:
ACT does the h1 relus and the out-copies (FD=1024, fused per-channel bout
bias); DVE does the residual scalar_tensor_tensor ops.

Output is staged in bf16 and stored with HWDGE (nc.scalar) DMAs into a
contiguous flush-major DRAM layout [NF*279, 4096] (8KB per partition row);
the host de-interleaves and upcasts.  bin/bh biases (zero in the graded net)
ride K=64 accumulating matmuls with the bias vector in row 0 of the
stationary against an all-ones moving operand.
"""
import numpy as np

from concourse import bacc, tile
import concourse.mybir as mybir
from concourse.bass_utils import run_bass_kernel_spmd

F32 = mybir.dt.float32
BF16 = mybir.dt.bfloat16
AF = mybir.ActivationFunctionType
ALU = mybir.AluOpType

B, C, H, W = 4, 64, 256, 256
NCORES = 8
NPX = (B * H * W) // NCORES          # 32768 pixels per core
NG = NPX // 2                        # 16384 per group (A/B)
T = 512                              # pixels per matmul tile
NT = NG // T                         # 32 pair-tiles per core
GT = 4                               # pair-tiles per flush chunk
NF = NT // GT                        # 8 chunks
FW = GT * 2 * T                      # staged columns per flush (4096)
OD = 279                             # output channels

SECS = ("co", "bp", "ins")
MO = {"co": 83, "bp": 68, "ins": 128}
OCH = {"co": (0, 83), "bp": (83, 151), "ins": (151, 279)}

# packed weight-tensor column layout (bf16).  Row-tiled stationaries are
# duplicated on both partition halves; bias-MM segments live in row 0 of
# partitions 0 and 64; out biases are per-partition columns in wpf (f32).
_W_COLS = {}
_c = 0
for _n in ("sin_co", "sin_bp", "sin_ins",
           "sl1_co", "sl1_bp", "sl1_ins", "sl2_co", "sl2_bp", "sl2_ins",
           "sout_ins"):
    _W_COLS[_n] = (_c, 128); _c += 128
_W_COLS["sout_co"] = (_c, MO["co"]); _c += MO["co"]
_W_COLS["sout_bp"] = (_c, MO["bp"]); _c += MO["bp"]
_W_COLS["ones"] = (_c, T); _c += T
for _n in ("bb_in_co", "bb_in_bp", "bb_in_ins",
           "bb_h1_co", "bb_h1_bp", "bb_h1_ins",
           "bb_h2_co", "bb_h2_bp", "bb_h2_ins"):
    _W_COLS[_n] = (_c, 128); _c += 128
WCOLS_TOTAL = _c

_last_results = None                 # test.py reads exec_time_ns from here
_cache = {}


def _bd(a, b):
    out = np.zeros((a.shape[0] + b.shape[0], a.shape[1] + b.shape[1]), np.float32)
    out[:a.shape[0], :a.shape[1]] = a
    out[a.shape[0]:, a.shape[1]:] = b
    return out


def _build(bin_nz: bool, bh_nz: bool):
    nc = bacc.Bacc("TRN2", target_bir_lowering=False, debug=False)

    xs_d = nc.dram_tensor("xs", [128, NG], BF16, kind="ExternalInput")
    wp_d = nc.dram_tensor("wp", [128, WCOLS_TOTAL], BF16, kind="ExternalInput")
    wpf_d = nc.dram_tensor("wpf", [128, 4], F32, kind="ExternalInput")
    out_d = nc.dram_tensor("out", [NF * OD, FW], BF16, kind="ExternalOutput")

    with tile.TileContext(nc) as tc:
        with tc.tile_pool(name="const", bufs=1) as cpool, \
             tc.tile_pool(name="xp", bufs=3) as xpool, \
             tc.tile_pool(name="hp", bufs=2) as hpool, \
             tc.tile_pool(name="op", bufs=2) as opool, \
             tc.tile_pool(name="ps", bufs=1, space="PSUM") as pspool:

            wp_t = cpool.tile([128, WCOLS_TOTAL], BF16, tag="wp")
            nc.sync.dma_start(out=wp_t[:], in_=wp_d.ap())
            wpf_t = cpool.tile([128, 4], F32, tag="wpf")
            nc.sync.dma_start(out=wpf_t[:], in_=wpf_d.ap())

            def wap(name, half=None):
                c0, n = _W_COLS[name]
                if half is None:
                    return wp_t[:, c0:c0 + n]
                return wp_t[half * 64:(half + 1) * 64, c0:c0 + n]

            _BCOL_IDX = {"co": 0, "bp": 1, "ins": 2}

            def bap(s):                  # out-bias column AP (f32)
                return wpf_t[0:MO[s], _BCOL_IDX[s]:_BCOL_IDX[s] + 1]

            def mm(out, stat, rhs, start, stop, tp=None):
                nc.tensor.matmul(out, stat, rhs, start=start, stop=stop,
                                 tile_position=tp)

            def bias_mm(p_cols, seg, stop):
                # p_cols[m, n] += seg[m]  (bias in row 0 of a [64,128]
                # stationary, all-ones moving operand, row-tile 0)
                mm(p_cols, wap(seg, 0), wap("ones", 0), False, stop, tp=(0, 0))

            def p2(name):            # 2-bank [128,1024] psum ring (bufs=3)
                return pspool.tile([128, 2 * T], F32, tag="p2", name=name,
                                   bufs=3)

            def p1(name):            # 1-bank [128,512] psum ring (bufs=2)
                return pspool.tile([128, T], F32, tag="p1", name=name,
                                   bufs=2)

            def load_x(f):
                x_t = xpool.tile([128, GT * T], BF16, tag="x", name=f"x_{f}")
                nc.sync.dma_start(
                    out=x_t[:],
                    in_=xs_d.ap()[:, f * GT * T:(f + 1) * GT * T])
                return x_t

            x_next = load_x(0)
            st = None
            for f in range(NF):
                x_t = x_next
                if f + 1 < NF:
                    x_next = load_x(f + 1)

                for tl in range(GT):
                    if tl == 0:
                        st = {s: opool.tile([MO[s], FW], BF16,
                                            tag="st" + s, name="st_" + s)
                              for s in SECS}
                    xg = x_t[:, tl * T:(tl + 1) * T]

                    # ---------------- in-proj ----------------
                    # co/bp: 64x128 row-tiled pairs (A rows 0-63, B 64-127,
                    # concurrent); ins: 128x128 block-diag dup.
                    pco = p2("pco")
                    pbp = p2("pbp")
                    pin = p1("pin")
                    for p, s in ((pco, "co"), (pbp, "bp")):
                        for g in (0, 1):
                            mm(p[:, g * T:(g + 1) * T], wap("sin_" + s, g),
                               xg[g * 64:(g + 1) * 64, :], True, not bin_nz,
                               tp=(g * 64, 0))
                            if bin_nz:
                                bias_mm(p[:, g * T:(g + 1) * T],
                                        "bb_in_" + s, True)
                    mm(pin[:], wap("sin_ins"), xg, True, not bin_nz)
                    if bin_nz:
                        bias_mm(pin[:], "bb_in_ins", True)

                    h1 = {"co": hpool.tile([128, 2 * T], BF16, tag="h1co",
                                           name="h1co"),
                          "bp": hpool.tile([128, 2 * T], BF16, tag="h1bp",
                                           name="h1bp"),
                          "ins": hpool.tile([128, T], BF16, tag="h1ins",
                                            name="h1ins")}
                    nc.scalar.activation(h1["co"][:], pco[:], AF.Relu)
                    nc.scalar.activation(h1["bp"][:], pbp[:], AF.Relu)
                    nc.scalar.activation(h1["ins"][:], pin[:], AF.Relu)

                    # ---------------- hidden layers (128x128 block-diag) ---
                    def hidden(l, h):
                        lco = p2("lco")
                        lbp = p2("lbp")
                        lin = p1("lin")
                        for p, s in ((lco, "co"), (lbp, "bp")):
                            for g in (0, 1):
                                mm(p[:, g * T:(g + 1) * T], wap(f"sl{l}_" + s),
                                   h[s][:, g * T:(g + 1) * T], True, not bh_nz)
                                if bh_nz:
                                    bias_mm(p[:, g * T:(g + 1) * T],
                                            f"bb_h{l}_" + s, True)
                        mm(lin[:], wap(f"sl{l}_ins"), h["ins"][:],
                           True, not bh_nz)
                        if bh_nz:
                            bias_mm(lin[:], f"bb_h{l}_ins", True)
                        hn = {"co": hpool.tile([128, 2 * T], BF16,
                                               tag=f"h{l + 1}co", name="hnco"),
                              "bp": hpool.tile([128, 2 * T], BF16,
                                               tag=f"h{l + 1}bp", name="hnbp"),
                              "ins": hpool.tile([128, T], BF16,
                                                tag=f"h{l + 1}ins",
                                                name="hnins")}
                        for s, p in (("co", lco), ("bp", lbp), ("ins", lin)):
                            nc.vector.scalar_tensor_tensor(
                                hn[s][:], p[:], 0.0, h[s][:], ALU.max, ALU.add)
                        return hn

                    h2 = hidden(1, h1)
                    h3 = hidden(2, h2)

                    # ---------------- out-proj ----------------
                    # co/bp: 128x128-mode block-diag [128, 83/68], bias fused
                    # into the ACT copy; ins: 64x128 row-tiled pair.
                    oco = pspool.tile([MO["co"], 2 * T], F32, tag="p2",
                                      name="oco", bufs=3)
                    obp = pspool.tile([MO["bp"], 2 * T], F32, tag="p2",
                                      name="obp", bufs=3)
                    oin = pspool.tile([128, 2 * T], F32, tag="p2",
                                      name="oin", bufs=3)
                    for g in (0, 1):
                        gc = slice(g * T, (g + 1) * T)
                        mm(oco[:, gc], wap("sout_co"), h3["co"][:, gc],
                           True, True)
                        mm(obp[:, gc], wap("sout_bp"), h3["bp"][:, gc],
                           True, True)
                        mm(oin[:, gc], wap("sout_ins", g),
                           h3["ins"][g * 64:(g + 1) * 64, :], True, True,
                           tp=(g * 64, 0))
                    col = slice(tl * 2 * T, (tl + 1) * 2 * T)
                    for s, p in (("co", oco), ("bp", obp), ("ins", oin)):
                        nc.scalar.activation(st[s][:, col], p[:],
                                             AF.Identity, bias=bap(s),
                                             scale=1.0)

                    # ---------------- flush ----------------
                    if tl == GT - 1:
                        oap = out_d.ap()
                        for s in SECS:
                            lo, hi = OCH[s]
                            nc.scalar.dma_start(
                                out=oap[f * OD + lo:f * OD + hi, :],
                                in_=st[s][:, :])

    nc.compile()
    return nc


def _prep_inputs(inputs):
    f32 = np.float32

    def wT(name):
        return np.ascontiguousarray(np.asarray(inputs[name], f32).T)

    m = {}
    co_in = np.concatenate([wT("cls_Win"), wT("obj_Win")], 1)   # [64, 128]
    bp_in = np.concatenate([wT("box_Win"), wT("pos_Win")], 1)
    m["sin_co"] = np.concatenate([co_in] * 2, 0)                # dup halves
    m["sin_bp"] = np.concatenate([bp_in] * 2, 0)
    m["sin_ins"] = _bd(wT("ins_Win"), wT("ins_Win"))
    for l in (1, 2):
        m[f"sl{l}_co"] = _bd(np.asarray(inputs["cls_Wh"][l - 1], f32).T,
                             np.asarray(inputs["obj_Wh"][l - 1], f32).T)
        m[f"sl{l}_bp"] = _bd(np.asarray(inputs["box_Wh"][l - 1], f32).T,
                             np.asarray(inputs["pos_Wh"][l - 1], f32).T)
        m[f"sl{l}_ins"] = _bd(np.asarray(inputs["ins_Wh"][l - 1], f32).T,
                              np.asarray(inputs["ins_Wh"][l - 1], f32).T)
    m["sout_co"] = _bd(wT("cls_Wout"), wT("obj_Wout"))          # [128, 83]
    m["sout_bp"] = _bd(wT("box_Wout"), wT("pos_Wout"))          # [128, 68]
    m["sout_ins"] = np.concatenate([wT("ins_Wout")] * 2, 0)     # [128, 128]

    def colv(v):
        return np.asarray(v, f32).reshape(-1)

    bseg = {}
    bseg["bb_in_co"] = np.concatenate([colv(inputs["cls_bin"]),
                                       colv(inputs["obj_bin"])])
    bseg["bb_in_bp"] = np.concatenate([colv(inputs["box_bin"]),
                                       colv(inputs["pos_bin"])])
    bseg["bb_in_ins"] = np.concatenate([colv(inputs["ins_bin"])] * 2)
    for l in (1, 2):
        bseg[f"bb_h{l}_co"] = np.concatenate([colv(inputs["cls_bh"][l - 1]),
                                              colv(inputs["obj_bh"][l - 1])])
        bseg[f"bb_h{l}_bp"] = np.concatenate([colv(inputs["box_bh"][l - 1]),
                                              colv(inputs["pos_bh"][l - 1])])
        bseg[f"bb_h{l}_ins"] = np.concatenate([colv(inputs["ins_bh"][l - 1])] * 2)
    bcol = {}
    bcol["co"] = np.concatenate([colv(inputs["cls_bout"]),
                                 colv(inputs["obj_bout"])])
    bcol["bp"] = np.concatenate([colv(inputs["box_bout"]),
                                 colv(inputs["pos_bout"])])
    bcol["ins"] = colv(inputs["ins_bout"])

    wp = np.zeros((128, WCOLS_TOTAL), f32)
    for name, (c0, n) in _W_COLS.items():
        if name == "ones":
            wp[:, c0:c0 + n] = 1.0
        elif name in m:
            v = m[name]
            wp[:v.shape[0], c0:c0 + n] = v
        else:
            wp[0, c0:c0 + n] = bseg[name]
            wp[64, c0:c0 + n] = bseg[name]
    wp = np.ascontiguousarray(wp)
    import ml_dtypes
    wpf = np.zeros((128, 4), f32)
    wpf[0:MO["co"], 0] = bcol["co"]
    wpf[0:MO["bp"], 1] = bcol["bp"]
    wpf[0:MO["ins"], 2] = bcol["ins"]
    wp = wp.astype(ml_dtypes.bfloat16)

    bin_nz = any(np.any(bseg["bb_in_" + s]) for s in SECS)
    bh_nz = any(np.any(bseg[f"bb_h{l}_" + s]) for s in SECS for l in (1, 2))

    x = np.asarray(inputs["x"], f32)
    in_maps = []
    for c in range(NCORES):
        b, hh = c // 2, c % 2
        xs = x[b, :, hh * 128:(hh + 1) * 128, :].reshape(64, NPX)
        xsr = np.ascontiguousarray(
            np.concatenate([xs[:, :NG], xs[:, NG:]],
                           axis=0)).astype(ml_dtypes.bfloat16)  # [128, NG]
        in_maps.append({"wp": wp, "xs": xsr, "wpf": wpf})
    return in_maps, (bin_nz, bh_nz)


def kernel(**inputs) -> np.ndarray:
    global _last_results
    in_maps, key = _prep_inputs(inputs)
    if key not in _cache:
        _cache[key] = _build(*key)
    nc = _cache[key]
    res = run_bass_kernel_spmd(nc, in_maps, core_ids=list(range(NCORES)))
    _last_results = res

    out = np.empty((B, OD, H, W), np.float32)
    for c in range(NCORES):
        b, hh = c // 2, c % 2
        o = np.asarray(res.results[c]["out"]).astype(np.float32)
        # staged layout: [NF, OD, GT, 2, T] -> per-group pixel-major
        blk = o.reshape(NF, OD, GT, 2, T)
        ga = blk[:, :, :, 0, :].transpose(1, 0, 2, 3).reshape(OD, NG)
        gb = blk[:, :, :, 1, :].transpose(1, 0, 2, 3).reshape(OD, NG)
        core = np.concatenate([ga, gb], axis=1)                 # [OD, NPX]
        out[b, :, hh * 128:(hh + 1) * 128, :] = core.reshape(OD, 128, W)
    return out


# revision 3
# speedup vs baseline: 1.1438x; 1.1438x over previous
"""Trainium2 Bass kernel for the 5-head detection tower (nn_DFD_10849087390476).

Network (per head h of 5): 1x1-conv tower on x [B,64,H,W]:
    h1 = relu(Win x + bin)
    h2 = h1 + relu(Wh0 h1 + bh0)
    h3 = h2 + relu(Wh1 h2 + bh1)
    out_h = Wout h3 + bout
Output = concat over heads: channels [cls 81, obj 2, box 4, pos 64, ins 128] = 279.

Sharding: data-parallel over (batch, H/2) -> 8 shards of 32768 pixels.
Per core the pixels form two 16384-px groups (A, B); a pair-tile is 512 px of
each.  Heads are paired on partition halves: co=(cls top, obj bottom),
bp=(box top, pos bottom), ins=(A top, B bottom).

All GEMMs have K=64.  in-proj and ins-out are 64x128 row-tiled matmul pairs
(tile_position (0,0)/(64,0)) running concurrently on the two PE row halves;
hidden layers and co/bp out are 128x128 block-diagonal.

PSUM is organized as two rings shared by every stage so the PE can run ahead
of evacuation: a 2-bank [128,1024] ring (bufs=3) for co/bp/out tiles and a
1-bank [128,512] ring (bufs=2) for ins tiles -- all 8 banks.  Evacuation:
ACT does the h1 relus and the out-copies (FD=1024, fused per-channel bout
bias); DVE does the residual scalar_tensor_tensor ops.

Output is staged in bf16 and stored with HWDGE (nc.scalar) DMAs into a
contiguous flush-major DRAM layout [NF*279, 4096] (8KB per partition row);
the host de-interleaves and upcasts.  bin/bh biases (zero in the graded net)
ride K=64 accumulating matmuls with the bias vector in row 0 of the
stationary against an all-ones moving operand.
"""
import numpy as np

from concourse import bacc, tile
import concourse.mybir as mybir
from concourse.bass_utils import run_bass_kernel_spmd

F32 = mybir.dt.float32
BF16 = mybir.dt.bfloat16
AF = mybir.ActivationFunctionType
ALU = mybir.AluOpType

B, C, H, W = 4, 64, 256, 256
NCORES = 8
NPX = (B * H * W) // NCORES          # 32768 pixels per core
NG = NPX // 2                        # 16384 per group (A/B)
T = 512                              # pixels per matmul tile
NT = NG // T                         # 32 pair-tiles per core
GT = 4                               # pair-tiles per flush chunk
NF = NT // GT                        # 8 chunks
FW = GT * 2 * T                      # staged columns per flush (4096)
OD = 279                             # output channels

SECS = ("co", "bp", "ins")
MO = {"co": 83, "bp": 68, "ins": 128}
OCH = {"co": (0, 83), "bp": (83, 151), "ins": (151, 279)}

# packed weight-tensor column layout (bf16).  Row-tiled stationaries are
# duplicated on both partition halves; bias-MM segments live in row 0 of
# partitions 0 and 64; out biases are per-partition columns in wpf (f32).
_W_COLS = {}
_c = 0
for _n in ("sin_co", "sin_bp", "sin_ins",
           "sl1_co", "sl1_bp", "sl1_ins", "sl2_co", "sl2_bp", "sl2_ins",
           "sout_ins"):
    _W_COLS[_n] = (_c, 128); _c += 128
_W_COLS["sout_co"] = (_c, MO["co"]); _c += MO["co"]
_W_COLS["sout_bp"] = (_c, MO["bp"]); _c += MO["bp"]
_W_COLS["ones"] = (_c, T); _c += T
for _n in ("bb_in_co", "bb_in_bp", "bb_in_ins",
           "bb_h1_co", "bb_h1_bp", "bb_h1_ins",
           "bb_h2_co", "bb_h2_bp", "bb_h2_ins"):
    _W_COLS[_n] = (_c, 128); _c += 128
WCOLS_TOTAL = _c

_last_results = None                 # test.py reads exec_time_ns from here
_cache = {}


def _bd(a, b):
    out = np.zeros((a.shape[0] + b.shape[0], a.shape[1] + b.shape[1]), np.float32)
    out[:a.shape[0], :a.shape[1]] = a
    out[a.shape[0]:, a.shape[1]:] = b
    return out


def _build(bin_nz: bool, bh_nz: bool):
    nc = bacc.Bacc("TRN2", target_bir_lowering=False, debug=False)

    xs_d = nc.dram_tensor("xs", [128, NG], BF16, kind="ExternalInput")
    wp_d = nc.dram_tensor("wp", [128, WCOLS_TOTAL], BF16, kind="ExternalInput")
    wpf_d = nc.dram_tensor("wpf", [128, 4], F32, kind="ExternalInput")
    out_d = nc.dram_tensor("out", [NF * OD, FW], BF16, kind="ExternalOutput")

    with tile.TileContext(nc) as tc:
        with tc.tile_pool(name="const", bufs=1) as cpool, \
             tc.tile_pool(name="xp", bufs=3) as xpool, \
             tc.tile_pool(name="hp", bufs=2) as hpool, \
             tc.tile_pool(name="op", bufs=2) as opool, \
             tc.tile_pool(name="ps", bufs=1, space="PSUM") as pspool:

            wp_t = cpool.tile([128, WCOLS_TOTAL], BF16, tag="wp")
            nc.sync.dma_start(out=wp_t[:], in_=wp_d.ap())
            wpf_t = cpool.tile([128, 4], F32, tag="wpf")
            nc.sync.dma_start(out=wpf_t[:], in_=wpf_d.ap())

            def wap(name, half=None):
                c0, n = _W_COLS[name]
                if half is None:
                    return wp_t[:, c0:c0 + n]
                return wp_t[half * 64:(half + 1) * 64, c0:c0 + n]

            _BCOL_IDX = {"co": 0, "bp": 1, "ins": 2}

            def bap(s):                  # out-bias column AP (f32)
                return wpf_t[0:MO[s], _BCOL_IDX[s]:_BCOL_IDX[s] + 1]

            def mm(out, stat, rhs, start, stop, tp=None):
                nc.tensor.matmul(out, stat, rhs, start=start, stop=stop,
                                 tile_position=tp)

            def bias_mm(p_cols, seg, stop):
                # p_cols[m, n] += seg[m]  (bias in row 0 of a [64,128]
                # stationary, all-ones moving operand, row-tile 0)
                mm(p_cols, wap(seg, 0), wap("ones", 0), False, stop, tp=(0, 0))

            def p2(name):            # 2-bank [128,1024] psum ring (bufs=3)
                return pspool.tile([128, 2 * T], F32, tag="p2", name=name,
                                   bufs=3)

            def p1(name):            # 1-bank [128,512] psum ring (bufs=2)
                return pspool.tile([128, T], F32, tag="p1", name=name,
                                   bufs=2)

            def load_x(f):
                x_t = xpool.tile([128, GT * T], BF16, tag="x", name=f"x_{f}")
                nc.sync.dma_start(
                    out=x_t[:],
                    in_=xs_d.ap()[:, f * GT * T:(f + 1) * GT * T])
                return x_t

            x_next = load_x(0)
            st = None
            for f in range(NF):
                x_t = x_next
                if f + 1 < NF:
                    x_next = load_x(f + 1)

                for tl in range(GT):
                    if tl == 0:
                        st = {s: opool.tile([MO[s], FW], BF16,
                                            tag="st" + s, name="st_" + s)
                              for s in SECS}
                    xg = x_t[:, tl * T:(tl + 1) * T]

                    # ---------------- in-proj ----------------
                    # co/bp: 64x128 row-tiled pairs (A rows 0-63, B 64-127,
                    # concurrent); ins: 128x128 block-diag dup.
                    pco = p2("pco")
                    pbp = p2("pbp")
                    pin = p1("pin")
                    for p, s in ((pco, "co"), (pbp, "bp")):
                        for g in (0, 1):
                            mm(p[:, g * T:(g + 1) * T], wap("sin_" + s, g),
                               xg[g * 64:(g + 1) * 64, :], True, not bin_nz,
                               tp=(g * 64, 0))
                            if bin_nz:
                                bias_mm(p[:, g * T:(g + 1) * T],
                                        "bb_in_" + s, True)
                    mm(pin[:], wap("sin_ins"), xg, True, not bin_nz)
                    if bin_nz:
                        bias_mm(pin[:], "bb_in_ins", True)

                    h1 = {"co": hpool.tile([128, 2 * T], BF16, tag="h1co",
                                           name="h1co"),
                          "bp": hpool.tile([128, 2 * T], BF16, tag="h1bp",
                                           name="h1bp"),
                          "ins": hpool.tile([128, T], BF16, tag="h1ins",
                                            name="h1ins")}
                    nc.scalar.activation(h1["co"][:], pco[:], AF.Relu)
                    nc.scalar.activation(h1["bp"][:], pbp[:], AF.Relu)
                    nc.scalar.activation(h1["ins"][:], pin[:], AF.Relu)

                    # ---------------- hidden layers (128x128 block-diag) ---
                    def hidden(l, h):
                        lco = p2("lco")
                        lbp = p2("lbp")
                        lin = p1("lin")
                        for p, s in ((lco, "co"), (lbp, "bp")):
                            for g in (0, 1):
                                mm(p[:, g * T:(g + 1) * T], wap(f"sl{l}_" + s),
                                   h[s][:, g * T:(g + 1) * T], True, not bh_nz)
                                if bh_nz:
                                    bias_mm(p[:, g * T:(g + 1) * T],
                                            f"bb_h{l}_" + s, True)
                        mm(lin[:], wap(f"sl{l}_ins"), h["ins"][:],
                           True, not bh_nz)
                        if bh_nz:
                            bias_mm(lin[:], f"bb_h{l}_ins", True)
                        hn = {"co": hpool.tile([128, 2 * T], BF16,
                                               tag=f"h{l + 1}co", name="hnco"),
                              "bp": hpool.tile([128, 2 * T], BF16,
                                               tag=f"h{l + 1}bp", name="hnbp"),
                              "ins": hpool.tile([128, T], BF16,
                                                tag=f"h{l + 1}ins",
                                                name="hnins")}
                        for s, p in (("co", lco), ("bp", lbp), ("ins", lin)):
                            nc.vector.scalar_tensor_tensor(
                                hn[s][:], p[:], 0.0, h[s][:], ALU.max, ALU.add)
                        return hn

                    h2 = hidden(1, h1)
                    h3 = hidden(2, h2)

                    # ---------------- out-proj ----------------
                    # co/bp: 128x128-mode block-diag [128, 83/68], bias fused
                    # into the ACT copy; ins: 64x128 row-tiled pair.
                    oco = pspool.tile([MO["co"], 2 * T], F32, tag="p2",
                                      name="oco", bufs=3)
                    obp = pspool.tile([MO["bp"], 2 * T], F32, tag="p2",
                                      name="obp", bufs=3)
                    oin = pspool.tile([128, 2 * T], F32, tag="p2",
                                      name="oin", bufs=3)
                    for g in (0, 1):
                        gc = slice(g * T, (g + 1) * T)
                        mm(oco[:, gc], wap("sout_co"), h3["co"][:, gc],
                           True, True)
                        mm(obp[:, gc], wap("sout_bp"), h3["bp"][:, gc],
                           True, True)
                        mm(oin[:, gc], wap("sout_ins", g),
                           h3["ins"][g * 64:(g + 1) * 64, :], True, True,
                           tp=(g * 64, 0))
                    col = slice(tl * 2 * T, (tl + 1) * 2 * T)
                    for s, p in (("co", oco), ("bp", obp), ("ins", oin)):
                        nc.scalar.activation(st[s][:, col], p[:],
                                             AF.Identity, bias=bap(s),
                                             scale=1.0)

                    # ---------------- flush ----------------
                    if tl == GT - 1:
                        oap = out_d.ap()
                        for s in SECS:
                            lo, hi = OCH[s]
                            nc.sync.dma_start(
                                out=oap[f * OD + lo:f * OD + hi, :],
                                in_=st[s][:, :])

    nc.compile()
    return nc


def _prep_inputs(inputs):
    f32 = np.float32

    def wT(name):
        return np.ascontiguousarray(np.asarray(inputs[name], f32).T)

    m = {}
    co_in = np.concatenate([wT("cls_Win"), wT("obj_Win")], 1)   # [64, 128]
    bp_in = np.concatenate([wT("box_Win"), wT("pos_Win")], 1)
    m["sin_co"] = np.concatenate([co_in] * 2, 0)                # dup halves
    m["sin_bp"] = np.concatenate([bp_in] * 2, 0)
    m["sin_ins"] = _bd(wT("ins_Win"), wT("ins_Win"))
    for l in (1, 2):
        m[f"sl{l}_co"] = _bd(np.asarray(inputs["cls_Wh"][l - 1], f32).T,
                             np.asarray(inputs["obj_Wh"][l - 1], f32).T)
        m[f"sl{l}_bp"] = _bd(np.asarray(inputs["box_Wh"][l - 1], f32).T,
                             np.asarray(inputs["pos_Wh"][l - 1], f32).T)
        m[f"sl{l}_ins"] = _bd(np.asarray(inputs["ins_Wh"][l - 1], f32).T,
                              np.asarray(inputs["ins_Wh"][l - 1], f32).T)
    m["sout_co"] = _bd(wT("cls_Wout"), wT("obj_Wout"))          # [128, 83]
    m["sout_bp"] = _bd(wT("box_Wout"), wT("pos_Wout"))          # [128, 68]
    m["sout_ins"] = np.concatenate([wT("ins_Wout")] * 2, 0)     # [128, 128]

    def colv(v):
        return np.asarray(v, f32).reshape(-1)

    bseg = {}
    bseg["bb_in_co"] = np.concatenate([colv(inputs["cls_bin"]),
                                       colv(inputs["obj_bin"])])
    bseg["bb_in_bp"] = np.concatenate([colv(inputs["box_bin"]),
                                       colv(inputs["pos_bin"])])
    bseg["bb_in_ins"] = np.concatenate([colv(inputs["ins_bin"])] * 2)
    for l in (1, 2):
        bseg[f"bb_h{l}_co"] = np.concatenate([colv(inputs["cls_bh"][l - 1]),
                                              colv(inputs["obj_bh"][l - 1])])
        bseg[f"bb_h{l}_bp"] = np.concatenate([colv(inputs["box_bh"][l - 1]),
                                              colv(inputs["pos_bh"][l - 1])])
        bseg[f"bb_h{l}_ins"] = np.concatenate([colv(inputs["ins_bh"][l - 1])] * 2)
    bcol = {}
    bcol["co"] = np.concatenate([colv(inputs["cls_bout"]),
                                 colv(inputs["obj_bout"])])
    bcol["bp"] = np.concatenate([colv(inputs["box_bout"]),
                                 colv(inputs["pos_bout"])])
    bcol["ins"] = colv(inputs["ins_bout"])

    wp = np.zeros((128, WCOLS_TOTAL), f32)
    for name, (c0, n) in _W_COLS.items():
        if name == "ones":
            wp[:, c0:c0 + n] = 1.0
        elif name in m:
            v = m[name]
            wp[:v.shape[0], c0:c0 + n] = v
        else:
            wp[0, c0:c0 + n] = bseg[name]
            wp[64, c0:c0 + n] = bseg[name]
    wp = np.ascontiguousarray(wp)
    import ml_dtypes
    wpf = np.zeros((128, 4), f32)
    wpf[0:MO["co"], 0] = bcol["co"]
    wpf[0:MO["bp"], 1] = bcol["bp"]
    wpf[0:MO["ins"], 2] = bcol["ins"]
    wp = wp.astype(ml_dtypes.bfloat16)

    bin_nz = any(np.any(bseg["bb_in_" + s]) for s in SECS)
    bh_nz = any(np.any(bseg[f"bb_h{l}_" + s]) for s in SECS for l in (1, 2))

    x = np.asarray(inputs["x"], f32)
    in_maps = []
    for c in range(NCORES):
        b, hh = c // 2, c % 2
        xs = x[b, :, hh * 128:(hh + 1) * 128, :].reshape(64, NPX)
        xsr = np.ascontiguousarray(
            np.concatenate([xs[:, :NG], xs[:, NG:]],
                           axis=0)).astype(ml_dtypes.bfloat16)  # [128, NG]
        in_maps.append({"wp": wp, "xs": xsr, "wpf": wpf})
    return in_maps, (bin_nz, bh_nz)


def kernel(**inputs) -> np.ndarray:
    global _last_results
    in_maps, key = _prep_inputs(inputs)
    if key not in _cache:
        _cache[key] = _build(*key)
    nc = _cache[key]
    res = run_bass_kernel_spmd(nc, in_maps, core_ids=list(range(NCORES)))
    _last_results = res

    out = np.empty((B, OD, H, W), np.float32)
    for c in range(NCORES):
        b, hh = c // 2, c % 2
        o = np.asarray(res.results[c]["out"]).astype(np.float32)
        # staged layout: [NF, OD, GT, 2, T] -> per-group pixel-major
        blk = o.reshape(NF, OD, GT, 2, T)
        ga = blk[:, :, :, 0, :].transpose(1, 0, 2, 3).reshape(OD, NG)
        gb = blk[:, :, :, 1, :].transpose(1, 0, 2, 3).reshape(OD, NG)
        core = np.concatenate([ga, gb], axis=1)                 # [OD, NPX]
        out[b, :, hh * 128:(hh + 1) * 128, :] = core.reshape(OD, 128, W)
    return out


# revision 5
# speedup vs baseline: 1.1456x; 1.0016x over previous
"""Trainium2 Bass kernel for the 5-head detection tower (nn_DFD_10849087390476).

Network (per head h of 5): 1x1-conv tower on x [B,64,H,W]:
    h1 = relu(Win x + bin)
    h2 = h1 + relu(Wh0 h1 + bh0)
    h3 = h2 + relu(Wh1 h2 + bh1)
    out_h = Wout h3 + bout
Output = concat over heads: channels [cls 81, obj 2, box 4, pos 64, ins 128] = 279.

Sharding: data-parallel over (batch, H/2) -> 8 shards of 32768 pixels.
Per core the pixels form two 16384-px groups (A, B); a pair-tile is 512 px of
each.  Heads are paired on partition halves: co=(cls top, obj bottom),
bp=(box top, pos bottom), ins=(A top, B bottom).

All GEMMs have K=64.  in-proj and ins-out are 64x128 row-tiled matmul pairs
(tile_position (0,0)/(64,0)) running concurrently on the two PE row halves;
hidden layers and co/bp out are 128x128 block-diagonal.

PSUM is organized as two rings shared by every stage so the PE can run ahead
of evacuation: a 2-bank [128,1024] ring (bufs=3) for co/bp/out tiles and a
1-bank [128,512] ring (bufs=2) for ins tiles -- all 8 banks.  Evacuation:
ACT does the h1 relus and the out-copies (FD=1024, fused per-channel bout
bias); DVE does the residual scalar_tensor_tensor ops.

Output is staged in bf16 and stored with HWDGE (nc.scalar) DMAs into a
contiguous flush-major DRAM layout [NF*279, 4096] (8KB per partition row);
the host de-interleaves and upcasts.  bin/bh biases (zero in the graded net)
ride K=64 accumulating matmuls with the bias vector in row 0 of the
stationary against an all-ones moving operand.
"""
import numpy as np

from concourse import bacc, tile
import concourse.mybir as mybir
from concourse.bass_utils import run_bass_kernel_spmd

F32 = mybir.dt.float32
BF16 = mybir.dt.bfloat16
AF = mybir.ActivationFunctionType
ALU = mybir.AluOpType

B, C, H, W = 4, 64, 256, 256
NCORES = 8
NPX = (B * H * W) // NCORES          # 32768 pixels per core
NG = NPX // 2                        # 16384 per group (A/B)
T = 512                              # pixels per matmul tile
NT = NG // T                         # 32 pair-tiles per core
GT = 4                               # pair-tiles per flush chunk
NF = NT // GT                        # 8 chunks
FW = GT * 2 * T                      # staged columns per flush (4096)
OD = 279                             # output channels

SECS = ("co", "bp", "ins")
MO = {"co": 83, "bp": 68, "ins": 128}
OCH = {"co": (0, 83), "bp": (83, 151), "ins": (151, 279)}

# packed weight-tensor column layout (bf16).  Row-tiled stationaries are
# duplicated on both partition halves; bias-MM segments live in row 0 of
# partitions 0 and 64; out biases are per-partition columns in wpf (f32).
_W_COLS = {}
_c = 0
for _n in ("sin_co", "sin_bp", "sin_ins",
           "sl1_co", "sl1_bp", "sl1_ins", "sl2_co", "sl2_bp", "sl2_ins",
           "sout_ins"):
    _W_COLS[_n] = (_c, 128); _c += 128
_W_COLS["sout_co"] = (_c, MO["co"]); _c += MO["co"]
_W_COLS["sout_bp"] = (_c, MO["bp"]); _c += MO["bp"]
_W_COLS["ones"] = (_c, T); _c += T
for _n in ("bb_in_co", "bb_in_bp", "bb_in_ins",
           "bb_h1_co", "bb_h1_bp", "bb_h1_ins",
           "bb_h2_co", "bb_h2_bp", "bb_h2_ins"):
    _W_COLS[_n] = (_c, 128); _c += 128
WCOLS_TOTAL = _c

_last_results = None                 # test.py reads exec_time_ns from here
_cache = {}


def _bd(a, b):
    out = np.zeros((a.shape[0] + b.shape[0], a.shape[1] + b.shape[1]), np.float32)
    out[:a.shape[0], :a.shape[1]] = a
    out[a.shape[0]:, a.shape[1]:] = b
    return out


def _build(bin_nz: bool, bh_nz: bool):
    nc = bacc.Bacc("TRN2", target_bir_lowering=False, debug=False)

    xs_d = nc.dram_tensor("xs", [128, NG], BF16, kind="ExternalInput")
    wp_d = nc.dram_tensor("wp", [128, WCOLS_TOTAL], BF16, kind="ExternalInput")
    wpf_d = nc.dram_tensor("wpf", [128, 4], F32, kind="ExternalInput")
    out_d = nc.dram_tensor("out", [NF * OD, FW], BF16, kind="ExternalOutput")

    with tile.TileContext(nc) as tc:
        with tc.tile_pool(name="const", bufs=1) as cpool, \
             tc.tile_pool(name="xp", bufs=3) as xpool, \
             tc.tile_pool(name="hp", bufs=2) as hpool, \
             tc.tile_pool(name="op", bufs=2) as opool, \
             tc.tile_pool(name="ps", bufs=1, space="PSUM") as pspool:

            wp_t = cpool.tile([128, WCOLS_TOTAL], BF16, tag="wp")
            nc.sync.dma_start(out=wp_t[:], in_=wp_d.ap())
            wpf_t = cpool.tile([128, 4], F32, tag="wpf")
            nc.sync.dma_start(out=wpf_t[:], in_=wpf_d.ap())

            def wap(name, half=None):
                c0, n = _W_COLS[name]
                if half is None:
                    return wp_t[:, c0:c0 + n]
                return wp_t[half * 64:(half + 1) * 64, c0:c0 + n]

            _BCOL_IDX = {"co": 0, "bp": 1, "ins": 2}

            def bap(s):                  # out-bias column AP (f32)
                return wpf_t[0:MO[s], _BCOL_IDX[s]:_BCOL_IDX[s] + 1]

            def mm(out, stat, rhs, start, stop, tp=None):
                nc.tensor.matmul(out, stat, rhs, start=start, stop=stop,
                                 tile_position=tp)

            def bias_mm(p_cols, seg, stop):
                # p_cols[m, n] += seg[m]  (bias in row 0 of a [64,128]
                # stationary, all-ones moving operand, row-tile 0)
                mm(p_cols, wap(seg, 0), wap("ones", 0), False, stop, tp=(0, 0))

            # PSUM rings: tagM holds the 2-bank co/bp tiles (8 allocs per
            # pair-tile vs 3 bufs -- coprime, so slot-reuse WARs always land
            # on an earlier natural dependency, never on the out-copies);
            # tagI holds the 1-bank ins tiles (5 allocs vs 2 bufs).
            def p2(name, rows=128):  # 2-bank [*,1024] psum ring (bufs=3)
                return pspool.tile([rows, 2 * T], F32, tag="p2", name=name,
                                   bufs=3)

            def p1(name):            # 1-bank [128,512] psum ring (bufs=2)
                return pspool.tile([128, T], F32, tag="p1", name=name,
                                   bufs=2)

            def load_x(f):
                x_t = xpool.tile([128, GT * T], BF16, tag="x", name=f"x_{f}")
                nc.sync.dma_start(
                    out=x_t[:],
                    in_=xs_d.ap()[:, f * GT * T:(f + 1) * GT * T])
                return x_t

            x_next = load_x(0)
            st = None
            for f in range(NF):
                x_t = x_next
                if f + 1 < NF:
                    x_next = load_x(f + 1)

                for tl in range(GT):
                    if tl == 0:
                        st = {s: opool.tile([MO[s], FW], BF16,
                                            tag="st" + s, name="st_" + s)
                              for s in SECS}
                    xg = x_t[:, tl * T:(tl + 1) * T]

                    # ---------------- in-proj ----------------
                    # co/bp: 64x128 row-tiled pairs (A rows 0-63, B 64-127,
                    # concurrent); ins: 128x128 block-diag dup.
                    pco = p2("pco")
                    pbp = p2("pbp")
                    pin = p1("pin")
                    for p, s in ((pco, "co"), (pbp, "bp")):
                        for g in (0, 1):
                            mm(p[:, g * T:(g + 1) * T], wap("sin_" + s, g),
                               xg[g * 64:(g + 1) * 64, :], True, not bin_nz,
                               tp=(g * 64, 0))
                            if bin_nz:
                                bias_mm(p[:, g * T:(g + 1) * T],
                                        "bb_in_" + s, True)
                    mm(pin[:], wap("sin_ins"), xg, True, not bin_nz)
                    if bin_nz:
                        bias_mm(pin[:], "bb_in_ins", True)

                    h1 = {"co": hpool.tile([128, 2 * T], BF16, tag="h1co",
                                           name="h1co"),
                          "bp": hpool.tile([128, 2 * T], BF16, tag="h1bp",
                                           name="h1bp"),
                          "ins": hpool.tile([128, T], BF16, tag="h1ins",
                                            name="h1ins")}
                    nc.scalar.activation(h1["co"][:], pco[:], AF.Relu)
                    nc.scalar.activation(h1["bp"][:], pbp[:], AF.Relu)
                    nc.scalar.activation(h1["ins"][:], pin[:], AF.Relu)

                    # ---------------- hidden layers (128x128 block-diag) ---
                    def hidden(l, h):
                        lco = p2("lco")
                        lbp = p2("lbp")
                        lin = p1("lin")
                        for p, s in ((lco, "co"), (lbp, "bp")):
                            for g in (0, 1):
                                mm(p[:, g * T:(g + 1) * T], wap(f"sl{l}_" + s),
                                   h[s][:, g * T:(g + 1) * T], True, not bh_nz)
                                if bh_nz:
                                    bias_mm(p[:, g * T:(g + 1) * T],
                                            f"bb_h{l}_" + s, True)
                        mm(lin[:], wap(f"sl{l}_ins"), h["ins"][:],
                           True, not bh_nz)
                        if bh_nz:
                            bias_mm(lin[:], f"bb_h{l}_ins", True)
                        hn = {"co": hpool.tile([128, 2 * T], BF16,
                                               tag=f"h{l + 1}co", name="hnco"),
                              "bp": hpool.tile([128, 2 * T], BF16,
                                               tag=f"h{l + 1}bp", name="hnbp"),
                              "ins": hpool.tile([128, T], BF16,
                                                tag=f"h{l + 1}ins",
                                                name="hnins")}
                        for s, p in (("co", lco), ("bp", lbp), ("ins", lin)):
                            nc.vector.scalar_tensor_tensor(
                                hn[s][:], p[:], 0.0, h[s][:], ALU.max, ALU.add)
                        return hn

                    h2 = hidden(1, h1)
                    h3 = hidden(2, h2)

                    # ---------------- out-proj ----------------
                    # co/bp: 128x128-mode block-diag [128, 83/68], bias fused
                    # into the ACT copy; ins: 64x128 row-tiled pair into two
                    # 1-bank tiles on the ins ring.
                    oco = p2("oco", rows=MO["co"])
                    obp = p2("obp", rows=MO["bp"])
                    oin = [p1("oin0"), p1("oin1")]
                    for g in (0, 1):
                        gc = slice(g * T, (g + 1) * T)
                        mm(oco[:, gc], wap("sout_co"), h3["co"][:, gc],
                           True, True)
                        mm(obp[:, gc], wap("sout_bp"), h3["bp"][:, gc],
                           True, True)
                        mm(oin[g][:], wap("sout_ins", g),
                           h3["ins"][g * 64:(g + 1) * 64, :], True, True,
                           tp=(g * 64, 0))
                    col = slice(tl * 2 * T, (tl + 1) * 2 * T)
                    for s, p in (("co", oco), ("bp", obp)):
                        nc.scalar.activation(st[s][:, col], p[:],
                                             AF.Identity, bias=bap(s),
                                             scale=1.0)
                    for g in (0, 1):
                        gcol = slice(tl * 2 * T + g * T,
                                     tl * 2 * T + (g + 1) * T)
                        nc.scalar.activation(st["ins"][:, gcol], oin[g][:],
                                             AF.Identity, bias=bap("ins"),
                                             scale=1.0)

                    # ---------------- flush ----------------
                    if tl == GT - 1:
                        oap = out_d.ap()
                        for s in SECS:
                            lo, hi = OCH[s]
                            nc.sync.dma_start(
                                out=oap[f * OD + lo:f * OD + hi, :],
                                in_=st[s][:, :])

    nc.compile()
    return nc


def _prep_inputs(inputs):
    f32 = np.float32

    def wT(name):
        return np.ascontiguousarray(np.asarray(inputs[name], f32).T)

    m = {}
    co_in = np.concatenate([wT("cls_Win"), wT("obj_Win")], 1)   # [64, 128]
    bp_in = np.concatenate([wT("box_Win"), wT("pos_Win")], 1)
    m["sin_co"] = np.concatenate([co_in] * 2, 0)                # dup halves
    m["sin_bp"] = np.concatenate([bp_in] * 2, 0)
    m["sin_ins"] = _bd(wT("ins_Win"), wT("ins_Win"))
    for l in (1, 2):
        m[f"sl{l}_co"] = _bd(np.asarray(inputs["cls_Wh"][l - 1], f32).T,
                             np.asarray(inputs["obj_Wh"][l - 1], f32).T)
        m[f"sl{l}_bp"] = _bd(np.asarray(inputs["box_Wh"][l - 1], f32).T,
                             np.asarray(inputs["pos_Wh"][l - 1], f32).T)
        m[f"sl{l}_ins"] = _bd(np.asarray(inputs["ins_Wh"][l - 1], f32).T,
                              np.asarray(inputs["ins_Wh"][l - 1], f32).T)
    m["sout_co"] = _bd(wT("cls_Wout"), wT("obj_Wout"))          # [128, 83]
    m["sout_bp"] = _bd(wT("box_Wout"), wT("pos_Wout"))          # [128, 68]
    m["sout_ins"] = np.concatenate([wT("ins_Wout")] * 2, 0)     # [128, 128]

    def colv(v):
        return np.asarray(v, f32).reshape(-1)

    bseg = {}
    bseg["bb_in_co"] = np.concatenate([colv(inputs["cls_bin"]),
                                       colv(inputs["obj_bin"])])
    bseg["bb_in_bp"] = np.concatenate([colv(inputs["box_bin"]),
                                       colv(inputs["pos_bin"])])
    bseg["bb_in_ins"] = np.concatenate([colv(inputs["ins_bin"])] * 2)
    for l in (1, 2):
        bseg[f"bb_h{l}_co"] = np.concatenate([colv(inputs["cls_bh"][l - 1]),
                                              colv(inputs["obj_bh"][l - 1])])
        bseg[f"bb_h{l}_bp"] = np.concatenate([colv(inputs["box_bh"][l - 1]),
                                              colv(inputs["pos_bh"][l - 1])])
        bseg[f"bb_h{l}_ins"] = np.concatenate([colv(inputs["ins_bh"][l - 1])] * 2)
    bcol = {}
    bcol["co"] = np.concatenate([colv(inputs["cls_bout"]),
                                 colv(inputs["obj_bout"])])
    bcol["bp"] = np.concatenate([colv(inputs["box_bout"]),
                                 colv(inputs["pos_bout"])])
    bcol["ins"] = colv(inputs["ins_bout"])

    wp = np.zeros((128, WCOLS_TOTAL), f32)
    for name, (c0, n) in _W_COLS.items():
        if name == "ones":
            wp[:, c0:c0 + n] = 1.0
        elif name in m:
            v = m[name]
            wp[:v.shape[0], c0:c0 + n] = v
        else:
            wp[0, c0:c0 + n] = bseg[name]
            wp[64, c0:c0 + n] = bseg[name]
    wp = np.ascontiguousarray(wp)
    import ml_dtypes
    wpf = np.zeros((128, 4), f32)
    wpf[0:MO["co"], 0] = bcol["co"]
    wpf[0:MO["bp"], 1] = bcol["bp"]
    wpf[0:MO["ins"], 2] = bcol["ins"]
    wp = wp.astype(ml_dtypes.bfloat16)

    bin_nz = any(np.any(bseg["bb_in_" + s]) for s in SECS)
    bh_nz = any(np.any(bseg[f"bb_h{l}_" + s]) for s in SECS for l in (1, 2))

    x = np.asarray(inputs["x"], f32)
    in_maps = []
    for c in range(NCORES):
        b, hh = c // 2, c % 2
        xs = x[b, :, hh * 128:(hh + 1) * 128, :].reshape(64, NPX)
        xsr = np.ascontiguousarray(
            np.concatenate([xs[:, :NG], xs[:, NG:]],
                           axis=0)).astype(ml_dtypes.bfloat16)  # [128, NG]
        in_maps.append({"wp": wp, "xs": xsr, "wpf": wpf})
    return in_maps, (bin_nz, bh_nz)


def kernel(**inputs) -> np.ndarray:
    global _last_results
    in_maps, key = _prep_inputs(inputs)
    if key not in _cache:
        _cache[key] = _build(*key)
    nc = _cache[key]
    res = run_bass_kernel_spmd(nc, in_maps, core_ids=list(range(NCORES)))
    _last_results = res

    out = np.empty((B, OD, H, W), np.float32)
    for c in range(NCORES):
        b, hh = c // 2, c % 2
        o = np.asarray(res.results[c]["out"]).astype(np.float32)
        # staged layout: [NF, OD, GT, 2, T] -> per-group pixel-major
        blk = o.reshape(NF, OD, GT, 2, T)
        ga = blk[:, :, :, 0, :].transpose(1, 0, 2, 3).reshape(OD, NG)
        gb = blk[:, :, :, 1, :].transpose(1, 0, 2, 3).reshape(OD, NG)
        core = np.concatenate([ga, gb], axis=1)                 # [OD, NPX]
        out[b, :, hh * 128:(hh + 1) * 128, :] = core.reshape(OD, 128, W)
    return out


# revision 6
# speedup vs baseline: 1.2643x; 1.1036x over previous
"""Trainium2 Bass kernel for the 5-head detection tower (nn_DFD_10849087390476).

Network (per head h of 5): 1x1-conv tower on x [B,64,H,W]:
    h1 = relu(Win x + bin)
    h2 = h1 + relu(Wh0 h1 + bh0)
    h3 = h2 + relu(Wh1 h2 + bh1)
    out_h = Wout h3 + bout
Output = concat over heads: channels [cls 81, obj 2, box 4, pos 64, ins 128] = 279.

Sharding: data-parallel over (batch, H/2) -> 8 shards of 32768 pixels.
Per core the pixels form two 16384-px groups (A, B); a pair-tile is 512 px of
each.  Heads are paired on partition halves: co=(cls top, obj bottom),
bp=(box top, pos bottom), ins=(A top, B bottom).

All GEMMs have K=64.  in-proj and ins-out are 64x128 row-tiled matmul pairs
(tile_position (0,0)/(64,0)) running concurrently on the two PE row halves;
hidden layers and co/bp out are 128x128 block-diagonal.

PSUM is organized as two rings shared by every stage so the PE can run ahead
of evacuation: a 2-bank [128,1024] ring (bufs=3) for co/bp/out tiles and a
1-bank [128,512] ring (bufs=2) for ins tiles -- all 8 banks.  Evacuation:
ACT does the h1 relus and the out-copies (FD=1024, fused per-channel bout
bias); DVE does the residual scalar_tensor_tensor ops.

Output is staged in bf16 and stored with HWDGE (nc.scalar) DMAs into a
contiguous flush-major DRAM layout [NF*279, 4096] (8KB per partition row);
the host de-interleaves and upcasts.  bin/bh biases (zero in the graded net)
ride K=64 accumulating matmuls with the bias vector in row 0 of the
stationary against an all-ones moving operand.
"""
import numpy as np

from concourse import bacc, tile
import concourse.mybir as mybir
from concourse.bass_utils import run_bass_kernel_spmd

F32 = mybir.dt.float32
BF16 = mybir.dt.bfloat16
AF = mybir.ActivationFunctionType
ALU = mybir.AluOpType

B, C, H, W = 4, 64, 256, 256
NCORES = 8
NPX = (B * H * W) // NCORES          # 32768 pixels per core
NG = NPX // 2                        # 16384 per group (A/B)
T = 512                              # pixels per matmul tile
NT = NG // T                         # 32 pair-tiles per core
GT = 4                               # pair-tiles per flush chunk
NF = NT // GT                        # 8 chunks
FW = GT * 2 * T                      # staged columns per flush (4096)
OD = 279                             # output channels

SECS = ("co", "bp", "ins")
MO = {"co": 83, "bp": 68, "ins": 128}
OCH = {"co": (0, 83), "bp": (83, 151), "ins": (151, 279)}

# packed weight-tensor column layout (bf16).  Row-tiled stationaries are
# duplicated on both partition halves; bias-MM segments live in row 0 of
# partitions 0 and 64; out biases are per-partition columns in wpf (f32).
_W_COLS = {}
_c = 0
for _n in ("sin_co", "sin_bp", "sin_ins",
           "sl1_co", "sl1_bp", "sl1_ins", "sl2_co", "sl2_bp", "sl2_ins",
           "sout_ins"):
    _W_COLS[_n] = (_c, 128); _c += 128
_W_COLS["sout_co"] = (_c, MO["co"]); _c += MO["co"]
_W_COLS["sout_bp"] = (_c, MO["bp"]); _c += MO["bp"]
_W_COLS["ones"] = (_c, T); _c += T
for _n in ("bb_in_co", "bb_in_bp", "bb_in_ins",
           "bb_h1_co", "bb_h1_bp", "bb_h1_ins",
           "bb_h2_co", "bb_h2_bp", "bb_h2_ins"):
    _W_COLS[_n] = (_c, 128); _c += 128
WCOLS_TOTAL = _c

_last_results = None                 # test.py reads exec_time_ns from here
_cache = {}


def _bd(a, b):
    out = np.zeros((a.shape[0] + b.shape[0], a.shape[1] + b.shape[1]), np.float32)
    out[:a.shape[0], :a.shape[1]] = a
    out[a.shape[0]:, a.shape[1]:] = b
    return out


def _build(bin_nz: bool, bh_nz: bool):
    nc = bacc.Bacc("TRN2", target_bir_lowering=False, debug=False)

    xs_d = nc.dram_tensor("xs", [128, NG], BF16, kind="ExternalInput")
    wp_d = nc.dram_tensor("wp", [128, WCOLS_TOTAL], BF16, kind="ExternalInput")
    wpf_d = nc.dram_tensor("wpf", [128, 4], F32, kind="ExternalInput")
    out_d = nc.dram_tensor("out", [NF * OD, FW], BF16, kind="ExternalOutput")

    with tile.TileContext(nc) as tc:
        with tc.tile_pool(name="const", bufs=1) as cpool, \
             tc.tile_pool(name="xp", bufs=3) as xpool, \
             tc.tile_pool(name="hp", bufs=2) as hpool, \
             tc.tile_pool(name="op", bufs=2) as opool, \
             tc.tile_pool(name="ps", bufs=1, space="PSUM") as pspool:

            wp_t = cpool.tile([128, WCOLS_TOTAL], BF16, tag="wp")
            nc.sync.dma_start(out=wp_t[:], in_=wp_d.ap())
            wpf_t = cpool.tile([128, 4], F32, tag="wpf")
            nc.sync.dma_start(out=wpf_t[:], in_=wpf_d.ap())

            def wap(name, half=None):
                c0, n = _W_COLS[name]
                if half is None:
                    return wp_t[:, c0:c0 + n]
                return wp_t[half * 64:(half + 1) * 64, c0:c0 + n]

            _BCOL_IDX = {"co": 0, "bp": 1, "ins": 2}

            def bap(s):                  # out-bias column AP (f32)
                return wpf_t[0:MO[s], _BCOL_IDX[s]:_BCOL_IDX[s] + 1]

            def mm(out, stat, rhs, start, stop, tp=None):
                nc.tensor.matmul(out, stat, rhs, start=start, stop=stop,
                                 tile_position=tp)

            def bias_mm(p_cols, seg, stop):
                # p_cols[m, n] += seg[m]  (bias in row 0 of a [64,128]
                # stationary, all-ones moving operand, row-tile 0)
                mm(p_cols, wap(seg, 0), wap("ones", 0), False, stop, tp=(0, 0))

            # PSUM rings: tagM holds the 2-bank co/bp tiles (8 allocs per
            # pair-tile vs 3 bufs -- coprime, so slot-reuse WARs always land
            # on an earlier natural dependency, never on the out-copies);
            # tagI holds the 1-bank ins tiles (5 allocs vs 2 bufs).
            def p2(name, rows=128):  # 2-bank [*,1024] psum ring (bufs=3)
                return pspool.tile([rows, 2 * T], F32, tag="p2", name=name,
                                   bufs=3)

            def p1(name):            # 1-bank [128,512] psum ring (bufs=2)
                return pspool.tile([128, T], F32, tag="p1", name=name,
                                   bufs=2)

            def load_x(f):
                x_t = xpool.tile([128, GT * T], BF16, tag="x", name=f"x_{f}")
                nc.sync.dma_start(
                    out=x_t[:],
                    in_=xs_d.ap()[:, f * GT * T:(f + 1) * GT * T])
                return x_t

            # --- stage emitters -------------------------------------------
            # The main loop is software-pipelined 3 deep: emission group g
            # runs in-proj of tile g, hidden-1 of tile g-1, and
            # hidden-2 + out-proj of tile g-2, so every intra-group chain is
            # at most 4 ops and ring WARs land on early/mid evacuations.

            def stage_in(t, x_t):
                xg = x_t[:, (t % GT) * T:(t % GT + 1) * T]
                pco = p2("pco")
                pbp = p2("pbp")
                pin = p1("pin")
                for p, s in ((pco, "co"), (pbp, "bp")):
                    for g in (0, 1):
                        mm(p[:, g * T:(g + 1) * T], wap("sin_" + s, g),
                           xg[g * 64:(g + 1) * 64, :], True, not bin_nz,
                           tp=(g * 64, 0))
                        if bin_nz:
                            bias_mm(p[:, g * T:(g + 1) * T],
                                    "bb_in_" + s, True)
                mm(pin[:], wap("sin_ins"), xg, True, not bin_nz)
                if bin_nz:
                    bias_mm(pin[:], "bb_in_ins", True)

                h1 = {"co": hpool.tile([128, 2 * T], BF16, tag="h1co",
                                       name="h1co"),
                      "bp": hpool.tile([128, 2 * T], BF16, tag="h1bp",
                                       name="h1bp"),
                      "ins": hpool.tile([128, T], BF16, tag="h1ins",
                                        name="h1ins")}
                nc.scalar.activation(h1["co"][:], pco[:], AF.Relu)
                nc.scalar.activation(h1["bp"][:], pbp[:], AF.Relu)
                nc.scalar.activation(h1["ins"][:], pin[:], AF.Relu)
                return h1

            def stage_hidden(l, h):
                lco = p2(f"l{l}co")
                lbp = p2(f"l{l}bp")
                lin = p1(f"l{l}in")
                for p, s in ((lco, "co"), (lbp, "bp")):
                    for g in (0, 1):
                        mm(p[:, g * T:(g + 1) * T], wap(f"sl{l}_" + s),
                           h[s][:, g * T:(g + 1) * T], True, not bh_nz)
                        if bh_nz:
                            bias_mm(p[:, g * T:(g + 1) * T],
                                    f"bb_h{l}_" + s, True)
                mm(lin[:], wap(f"sl{l}_ins"), h["ins"][:], True, not bh_nz)
                if bh_nz:
                    bias_mm(lin[:], f"bb_h{l}_ins", True)
                hn = {"co": hpool.tile([128, 2 * T], BF16, tag=f"h{l + 1}co",
                                       name="hnco"),
                      "bp": hpool.tile([128, 2 * T], BF16, tag=f"h{l + 1}bp",
                                       name="hnbp"),
                      "ins": hpool.tile([128, T], BF16, tag=f"h{l + 1}ins",
                                        name="hnins")}
                for s, p in (("co", lco), ("bp", lbp), ("ins", lin)):
                    nc.vector.scalar_tensor_tensor(
                        hn[s][:], p[:], 0.0, h[s][:], ALU.max, ALU.add)
                return hn

            def stage_out(t, h3, st):
                tl = t % GT
                oco = p2("oco", rows=MO["co"])
                obp = p2("obp", rows=MO["bp"])
                oin = [p1("oin0"), p1("oin1")]
                for g in (0, 1):
                    gc = slice(g * T, (g + 1) * T)
                    mm(oco[:, gc], wap("sout_co"), h3["co"][:, gc],
                       True, True)
                    mm(obp[:, gc], wap("sout_bp"), h3["bp"][:, gc],
                       True, True)
                    mm(oin[g][:], wap("sout_ins", g),
                       h3["ins"][g * 64:(g + 1) * 64, :], True, True,
                       tp=(g * 64, 0))
                col = slice(tl * 2 * T, (tl + 1) * 2 * T)
                for s, p in (("co", oco), ("bp", obp)):
                    nc.scalar.activation(st[s][:, col], p[:],
                                         AF.Identity, bias=bap(s),
                                         scale=1.0)
                for g in (0, 1):
                    gcol = slice(tl * 2 * T + g * T,
                                 tl * 2 * T + (g + 1) * T)
                    nc.scalar.activation(st["ins"][:, gcol], oin[g][:],
                                         AF.Identity, bias=bap("ins"),
                                         scale=1.0)
                if tl == GT - 1:
                    f = t // GT
                    oap = out_d.ap()
                    for s in SECS:
                        lo, hi = OCH[s]
                        nc.sync.dma_start(
                            out=oap[f * OD + lo:f * OD + hi, :],
                            in_=st[s][:, :])

            # --- software-pipelined main loop ------------------------------
            x_tiles = {0: load_x(0)}
            live = {}                 # tile -> {"h1"|"h2"|"st": ...}
            for g in range(NT + 2):
                if g < NT:
                    t = g
                    f, tl = t // GT, t % GT
                    if tl == 0 and f + 1 < NF:
                        x_tiles[f + 1] = load_x(f + 1)
                    if tl == 0:
                        st = {s: opool.tile([MO[s], FW], BF16,
                                            tag="st" + s, name="st_" + s)
                              for s in SECS}
                    live[t] = {"st": st}
                    live[t]["h1"] = stage_in(t, x_tiles[f])
                if g - 1 >= 0 and g - 1 < NT:
                    lv = live[g - 1]
                    lv["h2"] = stage_hidden(1, lv.pop("h1"))
                if g - 2 >= 0:
                    lv = live.pop(g - 2)
                    h3 = stage_hidden(2, lv.pop("h2"))
                    stage_out(g - 2, h3, lv.pop("st"))

    nc.compile()
    return nc


def _prep_inputs(inputs):
    f32 = np.float32

    def wT(name):
        return np.ascontiguousarray(np.asarray(inputs[name], f32).T)

    m = {}
    co_in = np.concatenate([wT("cls_Win"), wT("obj_Win")], 1)   # [64, 128]
    bp_in = np.concatenate([wT("box_Win"), wT("pos_Win")], 1)
    m["sin_co"] = np.concatenate([co_in] * 2, 0)                # dup halves
    m["sin_bp"] = np.concatenate([bp_in] * 2, 0)
    m["sin_ins"] = _bd(wT("ins_Win"), wT("ins_Win"))
    for l in (1, 2):
        m[f"sl{l}_co"] = _bd(np.asarray(inputs["cls_Wh"][l - 1], f32).T,
                             np.asarray(inputs["obj_Wh"][l - 1], f32).T)
        m[f"sl{l}_bp"] = _bd(np.asarray(inputs["box_Wh"][l - 1], f32).T,
                             np.asarray(inputs["pos_Wh"][l - 1], f32).T)
        m[f"sl{l}_ins"] = _bd(np.asarray(inputs["ins_Wh"][l - 1], f32).T,
                              np.asarray(inputs["ins_Wh"][l - 1], f32).T)
    m["sout_co"] = _bd(wT("cls_Wout"), wT("obj_Wout"))          # [128, 83]
    m["sout_bp"] = _bd(wT("box_Wout"), wT("pos_Wout"))          # [128, 68]
    m["sout_ins"] = np.concatenate([wT("ins_Wout")] * 2, 0)     # [128, 128]

    def colv(v):
        return np.asarray(v, f32).reshape(-1)

    bseg = {}
    bseg["bb_in_co"] = np.concatenate([colv(inputs["cls_bin"]),
                                       colv(inputs["obj_bin"])])
    bseg["bb_in_bp"] = np.concatenate([colv(inputs["box_bin"]),
                                       colv(inputs["pos_bin"])])
    bseg["bb_in_ins"] = np.concatenate([colv(inputs["ins_bin"])] * 2)
    for l in (1, 2):
        bseg[f"bb_h{l}_co"] = np.concatenate([colv(inputs["cls_bh"][l - 1]),
                                              colv(inputs["obj_bh"][l - 1])])
        bseg[f"bb_h{l}_bp"] = np.concatenate([colv(inputs["box_bh"][l - 1]),
                                              colv(inputs["pos_bh"][l - 1])])
        bseg[f"bb_h{l}_ins"] = np.concatenate([colv(inputs["ins_bh"][l - 1])] * 2)
    bcol = {}
    bcol["co"] = np.concatenate([colv(inputs["cls_bout"]),
                                 colv(inputs["obj_bout"])])
    bcol["bp"] = np.concatenate([colv(inputs["box_bout"]),
                                 colv(inputs["pos_bout"])])
    bcol["ins"] = colv(inputs["ins_bout"])

    wp = np.zeros((128, WCOLS_TOTAL), f32)
    for name, (c0, n) in _W_COLS.items():
        if name == "ones":
            wp[:, c0:c0 + n] = 1.0
        elif name in m:
            v = m[name]
            wp[:v.shape[0], c0:c0 + n] = v
        else:
            wp[0, c0:c0 + n] = bseg[name]
            wp[64, c0:c0 + n] = bseg[name]
    wp = np.ascontiguousarray(wp)
    import ml_dtypes
    wpf = np.zeros((128, 4), f32)
    wpf[0:MO["co"], 0] = bcol["co"]
    wpf[0:MO["bp"], 1] = bcol["bp"]
    wpf[0:MO["ins"], 2] = bcol["ins"]
    wp = wp.astype(ml_dtypes.bfloat16)

    bin_nz = any(np.any(bseg["bb_in_" + s]) for s in SECS)
    bh_nz = any(np.any(bseg[f"bb_h{l}_" + s]) for s in SECS for l in (1, 2))

    x = np.asarray(inputs["x"], f32)
    in_maps = []
    for c in range(NCORES):
        b, hh = c // 2, c % 2
        xs = x[b, :, hh * 128:(hh + 1) * 128, :].reshape(64, NPX)
        xsr = np.ascontiguousarray(
            np.concatenate([xs[:, :NG], xs[:, NG:]],
                           axis=0)).astype(ml_dtypes.bfloat16)  # [128, NG]
        in_maps.append({"wp": wp, "xs": xsr, "wpf": wpf})
    return in_maps, (bin_nz, bh_nz)


def kernel(**inputs) -> np.ndarray:
    global _last_results
    in_maps, key = _prep_inputs(inputs)
    if key not in _cache:
        _cache[key] = _build(*key)
    nc = _cache[key]
    res = run_bass_kernel_spmd(nc, in_maps, core_ids=list(range(NCORES)))
    _last_results = res

    out = np.empty((B, OD, H, W), np.float32)
    for c in range(NCORES):
        b, hh = c // 2, c % 2
        o = np.asarray(res.results[c]["out"]).astype(np.float32)
        # staged layout: [NF, OD, GT, 2, T] -> per-group pixel-major
        blk = o.reshape(NF, OD, GT, 2, T)
        ga = blk[:, :, :, 0, :].transpose(1, 0, 2, 3).reshape(OD, NG)
        gb = blk[:, :, :, 1, :].transpose(1, 0, 2, 3).reshape(OD, NG)
        core = np.concatenate([ga, gb], axis=1)                 # [OD, NPX]
        out[b, :, hh * 128:(hh + 1) * 128, :] = core.reshape(OD, 128, W)
    return out


# revision 7
# speedup vs baseline: 1.2667x; 1.0019x over previous
"""Trainium2 Bass kernel for the 5-head detection tower (nn_DFD_10849087390476).

Network (per head h of 5): 1x1-conv tower on x [B,64,H,W]:
    h1 = relu(Win x + bin)
    h2 = h1 + relu(Wh0 h1 + bh0)
    h3 = h2 + relu(Wh1 h2 + bh1)
    out_h = Wout h3 + bout
Output = concat over heads: channels [cls 81, obj 2, box 4, pos 64, ins 128] = 279.

Sharding: data-parallel over (batch, H/2) -> 8 shards of 32768 pixels.
Per core the pixels form two 16384-px groups (A, B); a pair-tile is 512 px of
each.  Heads are paired on partition halves: co=(cls top, obj bottom),
bp=(box top, pos bottom), ins=(A top, B bottom).

All GEMMs have K=64.  in-proj and ins-out are 64x128 row-tiled matmul pairs
(tile_position (0,0)/(64,0)) running concurrently on the two PE row halves;
hidden layers and co/bp out are 128x128 block-diagonal.

PSUM is organized as two rings shared by every stage so the PE can run ahead
of evacuation: a 2-bank [128,1024] ring (bufs=3) for co/bp/out tiles and a
1-bank [128,512] ring (bufs=2) for ins tiles -- all 8 banks.  Evacuation:
ACT does the h1 relus and the out-copies (FD=1024, fused per-channel bout
bias); DVE does the residual scalar_tensor_tensor ops.

Output is staged in bf16 and stored with HWDGE (nc.scalar) DMAs into a
contiguous flush-major DRAM layout [NF*279, 4096] (8KB per partition row);
the host de-interleaves and upcasts.  bin/bh biases (zero in the graded net)
ride K=64 accumulating matmuls with the bias vector in row 0 of the
stationary against an all-ones moving operand.
"""
import numpy as np

from concourse import bacc, tile
import concourse.mybir as mybir
from concourse.bass_utils import run_bass_kernel_spmd

F32 = mybir.dt.float32
BF16 = mybir.dt.bfloat16
AF = mybir.ActivationFunctionType
ALU = mybir.AluOpType

B, C, H, W = 4, 64, 256, 256
NCORES = 8
NPX = (B * H * W) // NCORES          # 32768 pixels per core
NG = NPX // 2                        # 16384 per group (A/B)
T = 512                              # pixels per matmul tile
NT = NG // T                         # 32 pair-tiles per core
GT = 4                               # pair-tiles per flush chunk
NF = NT // GT                        # 8 chunks
FW = GT * 2 * T                      # staged columns per flush (4096)
OD = 279                             # output channels

SECS = ("co", "bp", "ins")
MO = {"co": 83, "bp": 68, "ins": 128}
OCH = {"co": (0, 83), "bp": (83, 151), "ins": (151, 279)}

# packed weight-tensor column layout (bf16).  Row-tiled stationaries are
# duplicated on both partition halves; bias-MM segments live in row 0 of
# partitions 0 and 64; out biases are per-partition columns in wpf (f32).
_W_COLS = {}
_c = 0
for _n in ("sin_co", "sin_bp", "sin_ins",
           "sl1_co", "sl1_bp", "sl1_ins", "sl2_co", "sl2_bp", "sl2_ins",
           "sout_ins"):
    _W_COLS[_n] = (_c, 128); _c += 128
_W_COLS["sout_co"] = (_c, MO["co"]); _c += MO["co"]
_W_COLS["sout_bp"] = (_c, MO["bp"]); _c += MO["bp"]
_W_COLS["ones"] = (_c, T); _c += T
for _n in ("bb_in_co", "bb_in_bp", "bb_in_ins",
           "bb_h1_co", "bb_h1_bp", "bb_h1_ins",
           "bb_h2_co", "bb_h2_bp", "bb_h2_ins"):
    _W_COLS[_n] = (_c, 128); _c += 128
WCOLS_TOTAL = _c

_last_results = None                 # test.py reads exec_time_ns from here
_cache = {}


def _bd(a, b):
    out = np.zeros((a.shape[0] + b.shape[0], a.shape[1] + b.shape[1]), np.float32)
    out[:a.shape[0], :a.shape[1]] = a
    out[a.shape[0]:, a.shape[1]:] = b
    return out


def _build(bin_nz: bool, bh_nz: bool):
    nc = bacc.Bacc("TRN2", target_bir_lowering=False, debug=False)

    xs_d = nc.dram_tensor("xs", [128, NG], BF16, kind="ExternalInput")
    wp_d = nc.dram_tensor("wp", [128, WCOLS_TOTAL], BF16, kind="ExternalInput")
    wpf_d = nc.dram_tensor("wpf", [128, 4], F32, kind="ExternalInput")
    out_d = nc.dram_tensor("out", [NF * OD, FW], BF16, kind="ExternalOutput")

    with tile.TileContext(nc) as tc:
        with tc.tile_pool(name="const", bufs=1) as cpool, \
             tc.tile_pool(name="xp", bufs=3) as xpool, \
             tc.tile_pool(name="hp", bufs=2) as hpool, \
             tc.tile_pool(name="op", bufs=2) as opool, \
             tc.tile_pool(name="ps", bufs=1, space="PSUM") as pspool:

            wp_t = cpool.tile([128, WCOLS_TOTAL], BF16, tag="wp")
            nc.sync.dma_start(out=wp_t[:], in_=wp_d.ap())
            wpf_t = cpool.tile([128, 4], F32, tag="wpf")
            nc.sync.dma_start(out=wpf_t[:], in_=wpf_d.ap())

            def wap(name, half=None):
                c0, n = _W_COLS[name]
                if half is None:
                    return wp_t[:, c0:c0 + n]
                return wp_t[half * 64:(half + 1) * 64, c0:c0 + n]

            _BCOL_IDX = {"co": 0, "bp": 1, "ins": 2}

            def bap(s):                  # out-bias column AP (f32)
                return wpf_t[0:MO[s], _BCOL_IDX[s]:_BCOL_IDX[s] + 1]

            def mm(out, stat, rhs, start, stop, tp=None):
                nc.tensor.matmul(out, stat, rhs, start=start, stop=stop,
                                 tile_position=tp)

            def bias_mm(p_cols, seg, stop):
                # p_cols[m, n] += seg[m]  (bias in row 0 of a [64,128]
                # stationary, all-ones moving operand, row-tile 0)
                mm(p_cols, wap(seg, 0), wap("ones", 0), False, stop, tp=(0, 0))

            # PSUM rings: tagM holds the 2-bank co/bp tiles (8 allocs per
            # pair-tile vs 3 bufs -- coprime, so slot-reuse WARs always land
            # on an earlier natural dependency, never on the out-copies);
            # tagI holds the 1-bank ins tiles (5 allocs vs 2 bufs).
            def p2(name, rows=128):  # 2-bank [*,1024] psum ring (bufs=3)
                return pspool.tile([rows, 2 * T], F32, tag="p2", name=name,
                                   bufs=3)

            def p1(name):            # 1-bank [128,512] psum ring (bufs=2)
                return pspool.tile([128, T], F32, tag="p1", name=name,
                                   bufs=2)

            def load_x(f):
                x_t = xpool.tile([128, GT * T], BF16, tag="x", name=f"x_{f}")
                nc.sync.dma_start(
                    out=x_t[:],
                    in_=xs_d.ap()[:, f * GT * T:(f + 1) * GT * T])
                return x_t

            # --- stage emitters -------------------------------------------
            # The main loop is software-pipelined 3 deep: emission group g
            # runs in-proj of tile g, hidden-1 of tile g-1, and
            # hidden-2 + out-proj of tile g-2, so every intra-group chain is
            # at most 4 ops and ring WARs land on early/mid evacuations.

            def stage_in(t, x_t):
                xg = x_t[:, (t % GT) * T:(t % GT + 1) * T]
                pco = p2("pco")
                pbp = p2("pbp")
                pin = p1("pin")
                for p, s in ((pco, "co"), (pbp, "bp")):
                    for g in (0, 1):
                        mm(p[:, g * T:(g + 1) * T], wap("sin_" + s, g),
                           xg[g * 64:(g + 1) * 64, :], True, not bin_nz,
                           tp=(g * 64, 0))
                        if bin_nz:
                            bias_mm(p[:, g * T:(g + 1) * T],
                                    "bb_in_" + s, True)
                mm(pin[:], wap("sin_ins"), xg, True, not bin_nz)
                if bin_nz:
                    bias_mm(pin[:], "bb_in_ins", True)

                h1 = {"co": hpool.tile([128, 2 * T], BF16, tag="h1co",
                                       name="h1co"),
                      "bp": hpool.tile([128, 2 * T], BF16, tag="h1bp",
                                       name="h1bp"),
                      "ins": hpool.tile([128, T], BF16, tag="h1ins",
                                        name="h1ins")}
                nc.scalar.activation(h1["co"][:], pco[:], AF.Relu)
                nc.scalar.activation(h1["bp"][:], pbp[:], AF.Relu)
                nc.scalar.activation(h1["ins"][:], pin[:], AF.Relu)
                return h1

            def stage_hidden(l, h):
                lco = p2(f"l{l}co")
                lbp = p2(f"l{l}bp")
                lin = p1(f"l{l}in")
                for p, s in ((lco, "co"), (lbp, "bp")):
                    for g in (0, 1):
                        mm(p[:, g * T:(g + 1) * T], wap(f"sl{l}_" + s),
                           h[s][:, g * T:(g + 1) * T], True, not bh_nz)
                        if bh_nz:
                            bias_mm(p[:, g * T:(g + 1) * T],
                                    f"bb_h{l}_" + s, True)
                mm(lin[:], wap(f"sl{l}_ins"), h["ins"][:], True, not bh_nz)
                if bh_nz:
                    bias_mm(lin[:], f"bb_h{l}_ins", True)
                hn = {"co": hpool.tile([128, 2 * T], BF16, tag=f"h{l + 1}co",
                                       name="hnco"),
                      "bp": hpool.tile([128, 2 * T], BF16, tag=f"h{l + 1}bp",
                                       name="hnbp"),
                      "ins": hpool.tile([128, T], BF16, tag=f"h{l + 1}ins",
                                        name="hnins")}
                for s, p in (("co", lco), ("bp", lbp), ("ins", lin)):
                    nc.vector.scalar_tensor_tensor(
                        hn[s][:], p[:], 0.0, h[s][:], ALU.max, ALU.add)
                return hn

            def stage_out(t, h3, st):
                tl = t % GT
                oco = p2("oco", rows=MO["co"])
                obp = p2("obp", rows=MO["bp"])
                oin = [p1("oin0"), p1("oin1")]
                for g in (0, 1):
                    gc = slice(g * T, (g + 1) * T)
                    mm(oco[:, gc], wap("sout_co"), h3["co"][:, gc],
                       True, True)
                    mm(obp[:, gc], wap("sout_bp"), h3["bp"][:, gc],
                       True, True)
                    mm(oin[g][:], wap("sout_ins", g),
                       h3["ins"][g * 64:(g + 1) * 64, :], True, True,
                       tp=(g * 64, 0))
                col = slice(tl * 2 * T, (tl + 1) * 2 * T)
                for s, p in (("co", oco), ("bp", obp)):
                    nc.scalar.activation(st[s][:, col], p[:],
                                         AF.Identity, bias=bap(s),
                                         scale=1.0)
                for g in (0, 1):
                    gcol = slice(tl * 2 * T + g * T,
                                 tl * 2 * T + (g + 1) * T)
                    nc.scalar.activation(st["ins"][:, gcol], oin[g][:],
                                         AF.Identity, bias=bap("ins"),
                                         scale=1.0)
                if tl == GT - 1:
                    f = t // GT
                    oap = out_d.ap()
                    for s in SECS:
                        lo, hi = OCH[s]
                        nc.sync.dma_start(
                            out=oap[f * OD + lo:f * OD + hi, :],
                            in_=st[s][:, :])

            # --- software-pipelined main loop (4-deep stage skew) ----------
            x_tiles = {0: load_x(0)}
            live = {}                 # tile -> {"h1"|"h2"|"h3"|"st": ...}
            for g in range(NT + 3):
                if g < NT:
                    t = g
                    f, tl = t // GT, t % GT
                    if tl == 0 and f + 1 < NF:
                        x_tiles[f + 1] = load_x(f + 1)
                    if tl == 0:
                        st = {s: opool.tile([MO[s], FW], BF16,
                                            tag="st" + s, name="st_" + s)
                              for s in SECS}
                    live[t] = {"st": st}
                    live[t]["h1"] = stage_in(t, x_tiles[f])
                if 0 <= g - 1 < NT:
                    lv = live[g - 1]
                    lv["h2"] = stage_hidden(1, lv.pop("h1"))
                if 0 <= g - 2 < NT:
                    lv = live[g - 2]
                    lv["h3"] = stage_hidden(2, lv.pop("h2"))
                if g - 3 >= 0:
                    lv = live.pop(g - 3)
                    stage_out(g - 3, lv.pop("h3"), lv.pop("st"))

    nc.compile()
    return nc


def _prep_inputs(inputs):
    f32 = np.float32

    def wT(name):
        return np.ascontiguousarray(np.asarray(inputs[name], f32).T)

    m = {}
    co_in = np.concatenate([wT("cls_Win"), wT("obj_Win")], 1)   # [64, 128]
    bp_in = np.concatenate([wT("box_Win"), wT("pos_Win")], 1)
    m["sin_co"] = np.concatenate([co_in] * 2, 0)                # dup halves
    m["sin_bp"] = np.concatenate([bp_in] * 2, 0)
    m["sin_ins"] = _bd(wT("ins_Win"), wT("ins_Win"))
    for l in (1, 2):
        m[f"sl{l}_co"] = _bd(np.asarray(inputs["cls_Wh"][l - 1], f32).T,
                             np.asarray(inputs["obj_Wh"][l - 1], f32).T)
        m[f"sl{l}_bp"] = _bd(np.asarray(inputs["box_Wh"][l - 1], f32).T,
                             np.asarray(inputs["pos_Wh"][l - 1], f32).T)
        m[f"sl{l}_ins"] = _bd(np.asarray(inputs["ins_Wh"][l - 1], f32).T,
                              np.asarray(inputs["ins_Wh"][l - 1], f32).T)
    m["sout_co"] = _bd(wT("cls_Wout"), wT("obj_Wout"))          # [128, 83]
    m["sout_bp"] = _bd(wT("box_Wout"), wT("pos_Wout"))          # [128, 68]
    m["sout_ins"] = np.concatenate([wT("ins_Wout")] * 2, 0)     # [128, 128]

    def colv(v):
        return np.asarray(v, f32).reshape(-1)

    bseg = {}
    bseg["bb_in_co"] = np.concatenate([colv(inputs["cls_bin"]),
                                       colv(inputs["obj_bin"])])
    bseg["bb_in_bp"] = np.concatenate([colv(inputs["box_bin"]),
                                       colv(inputs["pos_bin"])])
    bseg["bb_in_ins"] = np.concatenate([colv(inputs["ins_bin"])] * 2)
    for l in (1, 2):
        bseg[f"bb_h{l}_co"] = np.concatenate([colv(inputs["cls_bh"][l - 1]),
                                              colv(inputs["obj_bh"][l - 1])])
        bseg[f"bb_h{l}_bp"] = np.concatenate([colv(inputs["box_bh"][l - 1]),
                                              colv(inputs["pos_bh"][l - 1])])
        bseg[f"bb_h{l}_ins"] = np.concatenate([colv(inputs["ins_bh"][l - 1])] * 2)
    bcol = {}
    bcol["co"] = np.concatenate([colv(inputs["cls_bout"]),
                                 colv(inputs["obj_bout"])])
    bcol["bp"] = np.concatenate([colv(inputs["box_bout"]),
                                 colv(inputs["pos_bout"])])
    bcol["ins"] = colv(inputs["ins_bout"])

    wp = np.zeros((128, WCOLS_TOTAL), f32)
    for name, (c0, n) in _W_COLS.items():
        if name == "ones":
            wp[:, c0:c0 + n] = 1.0
        elif name in m:
            v = m[name]
            wp[:v.shape[0], c0:c0 + n] = v
        else:
            wp[0, c0:c0 + n] = bseg[name]
            wp[64, c0:c0 + n] = bseg[name]
    wp = np.ascontiguousarray(wp)
    import ml_dtypes
    wpf = np.zeros((128, 4), f32)
    wpf[0:MO["co"], 0] = bcol["co"]
    wpf[0:MO["bp"], 1] = bcol["bp"]
    wpf[0:MO["ins"], 2] = bcol["ins"]
    wp = wp.astype(ml_dtypes.bfloat16)

    bin_nz = any(np.any(bseg["bb_in_" + s]) for s in SECS)
    bh_nz = any(np.any(bseg[f"bb_h{l}_" + s]) for s in SECS for l in (1, 2))

    x = np.asarray(inputs["x"], f32)
    in_maps = []
    for c in range(NCORES):
        b, hh = c // 2, c % 2
        xs = x[b, :, hh * 128:(hh + 1) * 128, :].reshape(64, NPX)
        xsr = np.ascontiguousarray(
            np.concatenate([xs[:, :NG], xs[:, NG:]],
                           axis=0)).astype(ml_dtypes.bfloat16)  # [128, NG]
        in_maps.append({"wp": wp, "xs": xsr, "wpf": wpf})
    return in_maps, (bin_nz, bh_nz)


def kernel(**inputs) -> np.ndarray:
    global _last_results
    in_maps, key = _prep_inputs(inputs)
    if key not in _cache:
        _cache[key] = _build(*key)
    nc = _cache[key]
    res = run_bass_kernel_spmd(nc, in_maps, core_ids=list(range(NCORES)))
    _last_results = res

    out = np.empty((B, OD, H, W), np.float32)
    for c in range(NCORES):
        b, hh = c // 2, c % 2
        o = np.asarray(res.results[c]["out"]).astype(np.float32)
        # staged layout: [NF, OD, GT, 2, T] -> per-group pixel-major
        blk = o.reshape(NF, OD, GT, 2, T)
        ga = blk[:, :, :, 0, :].transpose(1, 0, 2, 3).reshape(OD, NG)
        gb = blk[:, :, :, 1, :].transpose(1, 0, 2, 3).reshape(OD, NG)
        core = np.concatenate([ga, gb], axis=1)                 # [OD, NPX]
        out[b, :, hh * 128:(hh + 1) * 128, :] = core.reshape(OD, 128, W)
    return out


# revision 9
# speedup vs baseline: 1.5742x; 1.2427x over previous
"""Trainium2 Bass kernel for the 5-head detection tower (nn_DFD_10849087390476).

Network (per head h of 5): 1x1-conv tower on x [B,64,H,W]:
    h1 = relu(Win x + bin)
    h2 = h1 + relu(Wh0 h1 + bh0)
    h3 = h2 + relu(Wh1 h2 + bh1)
    out_h = Wout h3 + bout
Output = concat over heads: channels [cls 81, obj 2, box 4, pos 64, ins 128] = 279.

Sharding: data-parallel over (batch, H/2) -> 8 shards of 32768 pixels.
Per core the pixels form two 16384-px groups (A, B); a pair-tile is 512 px of
each.  Heads are paired on partition halves: co=(cls top, obj bottom),
bp=(box top, pos bottom), ins=(A top, B bottom).

All GEMMs have K=64.  in-proj and ins-out are 64x128 row-tiled matmul pairs
(tile_position (0,0)/(64,0)) running concurrently on the two PE row halves;
hidden layers and co/bp out are 128x128 block-diagonal.

PSUM is organized as two rings shared by every stage so the PE can run ahead
of evacuation: a 2-bank [128,1024] ring (bufs=3) for co/bp/out tiles and a
1-bank [128,512] ring (bufs=2) for ins tiles -- all 8 banks.  Evacuation:
ACT does the h1 relus and the out-copies (FD=1024, fused per-channel bout
bias); DVE does the residual scalar_tensor_tensor ops.

Output is staged in bf16 and stored with HWDGE (nc.scalar) DMAs into a
contiguous flush-major DRAM layout [NF*279, 4096] (8KB per partition row);
the host de-interleaves and upcasts.  bin/bh biases (zero in the graded net)
ride K=64 accumulating matmuls with the bias vector in row 0 of the
stationary against an all-ones moving operand.
"""
import numpy as np

from concourse import bacc, tile
import concourse.mybir as mybir
from concourse.bass_utils import run_bass_kernel_spmd

F32 = mybir.dt.float32
BF16 = mybir.dt.bfloat16
AF = mybir.ActivationFunctionType
ALU = mybir.AluOpType

B, C, H, W = 4, 64, 256, 256
NCORES = 8
NPX = (B * H * W) // NCORES          # 32768 pixels per core
NG = NPX // 2                        # 16384 per group (A/B)
T = 512                              # pixels per matmul tile
NT = NG // T                         # 32 pair-tiles per core
GT = 4                               # pair-tiles per flush chunk
NF = NT // GT                        # 8 chunks
FW = GT * 2 * T                      # staged columns per flush (4096)
OD = 279                             # output channels

SECS = ("co", "bp", "ins")
MO = {"co": 83, "bp": 68, "ins": 128}
OCH = {"co": (0, 83), "bp": (83, 151), "ins": (151, 279)}

# packed weight-tensor column layout (bf16).  Row-tiled stationaries are
# duplicated on both partition halves; bias-MM segments live in row 0 of
# partitions 0 and 64; out biases are per-partition columns in wpf (f32).
_W_COLS = {}
_c = 0
for _n in ("sin_co", "sin_bp", "sin_ins",
           "sl1_co", "sl1_bp", "sl1_ins", "sl2_co", "sl2_bp", "sl2_ins",
           "sout_ins"):
    _W_COLS[_n] = (_c, 128); _c += 128
_W_COLS["sout_co"] = (_c, MO["co"]); _c += MO["co"]
_W_COLS["sout_bp"] = (_c, MO["bp"]); _c += MO["bp"]
_W_COLS["ones"] = (_c, T); _c += T
for _n in ("bb_in_co", "bb_in_bp", "bb_in_ins",
           "bb_h1_co", "bb_h1_bp", "bb_h1_ins",
           "bb_h2_co", "bb_h2_bp", "bb_h2_ins"):
    _W_COLS[_n] = (_c, 128); _c += 128
WCOLS_TOTAL = _c

_last_results = None                 # test.py reads exec_time_ns from here
_cache = {}


def _bd(a, b):
    out = np.zeros((a.shape[0] + b.shape[0], a.shape[1] + b.shape[1]), np.float32)
    out[:a.shape[0], :a.shape[1]] = a
    out[a.shape[0]:, a.shape[1]:] = b
    return out


def _build(bin_nz: bool, bh_nz: bool):
    nc = bacc.Bacc("TRN2", target_bir_lowering=False, debug=False)

    xs_d = nc.dram_tensor("xs", [128, NG], BF16, kind="ExternalInput")
    wp_d = nc.dram_tensor("wp", [128, WCOLS_TOTAL], BF16, kind="ExternalInput")
    wpf_d = nc.dram_tensor("wpf", [128, 4], F32, kind="ExternalInput")
    out_d = nc.dram_tensor("out", [NF * OD, FW], BF16, kind="ExternalOutput")

    with tile.TileContext(nc) as tc:
        with tc.tile_pool(name="const", bufs=1) as cpool, \
             tc.tile_pool(name="xp", bufs=3) as xpool, \
             tc.tile_pool(name="hp", bufs=2) as hpool, \
             tc.tile_pool(name="op", bufs=3) as opool, \
             tc.tile_pool(name="ps", bufs=1, space="PSUM") as pspool:

            wp_t = cpool.tile([128, WCOLS_TOTAL], BF16, tag="wp")
            nc.sync.dma_start(out=wp_t[:], in_=wp_d.ap())
            wpf_t = cpool.tile([128, 4], F32, tag="wpf")
            nc.sync.dma_start(out=wpf_t[:], in_=wpf_d.ap())

            def wap(name, half=None):
                c0, n = _W_COLS[name]
                if half is None:
                    return wp_t[:, c0:c0 + n]
                return wp_t[half * 64:(half + 1) * 64, c0:c0 + n]

            _BCOL_IDX = {"co": 0, "bp": 1, "ins": 2}

            def bap(s):                  # out-bias column AP (f32)
                return wpf_t[0:MO[s], _BCOL_IDX[s]:_BCOL_IDX[s] + 1]

            def mm(out, stat, rhs, start, stop, tp=None):
                nc.tensor.matmul(out, stat, rhs, start=start, stop=stop,
                                 tile_position=tp)

            def bias_mm(p_cols, seg, stop):
                # p_cols[m, n] += seg[m]  (bias in row 0 of a [64,128]
                # stationary, all-ones moving operand, row-tile 0)
                mm(p_cols, wap(seg, 0), wap("ones", 0), False, stop, tp=(0, 0))

            # PSUM rings: tagM holds the 2-bank co/bp tiles (8 allocs per
            # pair-tile vs 3 bufs -- coprime, so slot-reuse WARs always land
            # on an earlier natural dependency, never on the out-copies);
            # tagI holds the 1-bank ins tiles (5 allocs vs 2 bufs).
            def p2(name, rows=128):  # 2-bank [*,1024] psum ring (bufs=3)
                return pspool.tile([rows, 2 * T], F32, tag="p2", name=name,
                                   bufs=3)

            def p1(name):            # 1-bank [128,512] psum ring (bufs=2)
                return pspool.tile([128, T], F32, tag="p1", name=name,
                                   bufs=2)

            def load_x(f):
                x_t = xpool.tile([128, GT * T], BF16, tag="x", name=f"x_{f}")
                nc.sync.dma_start(
                    out=x_t[:],
                    in_=xs_d.ap()[:, f * GT * T:(f + 1) * GT * T])
                return x_t

            # --- stage emitters -------------------------------------------
            # The main loop is software-pipelined 3 deep: emission group g
            # runs in-proj of tile g, hidden-1 of tile g-1, and
            # hidden-2 + out-proj of tile g-2, so every intra-group chain is
            # at most 4 ops and ring WARs land on early/mid evacuations.

            def stage_in(t, x_t):
                xg = x_t[:, (t % GT) * T:(t % GT + 1) * T]
                pco = p2("pco")
                pbp = p2("pbp")
                pin = p1("pin")
                for p, s in ((pco, "co"), (pbp, "bp")):
                    for g in (0, 1):
                        mm(p[:, g * T:(g + 1) * T], wap("sin_" + s, g),
                           xg[g * 64:(g + 1) * 64, :], True, not bin_nz,
                           tp=(g * 64, 0))
                        if bin_nz:
                            bias_mm(p[:, g * T:(g + 1) * T],
                                    "bb_in_" + s, True)
                mm(pin[:], wap("sin_ins"), xg, True, not bin_nz)
                if bin_nz:
                    bias_mm(pin[:], "bb_in_ins", True)

                h1 = {"co": hpool.tile([128, 2 * T], BF16, tag="h1co",
                                       name="h1co"),
                      "bp": hpool.tile([128, 2 * T], BF16, tag="h1bp",
                                       name="h1bp"),
                      "ins": hpool.tile([128, T], BF16, tag="h1ins",
                                        name="h1ins")}
                nc.scalar.activation(h1["co"][:], pco[:], AF.Relu)
                nc.scalar.activation(h1["bp"][:], pbp[:], AF.Relu)
                nc.scalar.activation(h1["ins"][:], pin[:], AF.Relu)
                return h1

            def stage_hidden(l, h):
                lco = p2(f"l{l}co")
                lbp = p2(f"l{l}bp")
                lin = p1(f"l{l}in")
                for p, s in ((lco, "co"), (lbp, "bp")):
                    for g in (0, 1):
                        mm(p[:, g * T:(g + 1) * T], wap(f"sl{l}_" + s),
                           h[s][:, g * T:(g + 1) * T], True, not bh_nz)
                        if bh_nz:
                            bias_mm(p[:, g * T:(g + 1) * T],
                                    f"bb_h{l}_" + s, True)
                mm(lin[:], wap(f"sl{l}_ins"), h["ins"][:], True, not bh_nz)
                if bh_nz:
                    bias_mm(lin[:], f"bb_h{l}_ins", True)
                hn = {"co": hpool.tile([128, 2 * T], BF16, tag=f"h{l + 1}co",
                                       name="hnco"),
                      "bp": hpool.tile([128, 2 * T], BF16, tag=f"h{l + 1}bp",
                                       name="hnbp"),
                      "ins": hpool.tile([128, T], BF16, tag=f"h{l + 1}ins",
                                        name="hnins")}
                for s, p in (("co", lco), ("bp", lbp), ("ins", lin)):
                    nc.vector.scalar_tensor_tensor(
                        hn[s][:], p[:], 0.0, h[s][:], ALU.max, ALU.add)
                return hn

            def stage_out(t, h3, st):
                tl = t % GT
                oco = p2("oco", rows=MO["co"])
                obp = p2("obp", rows=MO["bp"])
                oin = [p1("oin0"), p1("oin1")]
                for g in (0, 1):
                    gc = slice(g * T, (g + 1) * T)
                    mm(oco[:, gc], wap("sout_co"), h3["co"][:, gc],
                       True, True)
                    mm(obp[:, gc], wap("sout_bp"), h3["bp"][:, gc],
                       True, True)
                    mm(oin[g][:], wap("sout_ins", g),
                       h3["ins"][g * 64:(g + 1) * 64, :], True, True,
                       tp=(g * 64, 0))
                col = slice(tl * 2 * T, (tl + 1) * 2 * T)
                for s, p in (("co", oco), ("bp", obp)):
                    nc.scalar.activation(st[s][:, col], p[:],
                                         AF.Identity, bias=bap(s),
                                         scale=1.0)
                for g in (0, 1):
                    gcol = slice(tl * 2 * T + g * T,
                                 tl * 2 * T + (g + 1) * T)
                    nc.scalar.activation(st["ins"][:, gcol], oin[g][:],
                                         AF.Identity, bias=bap("ins"),
                                         scale=1.0)
                if tl == GT - 1:
                    # SWDGE stores: gpsimd is otherwise idle, its descriptor
                    # swizzle spreads evenly over all 16 SDMA engines, and it
                    # keeps the store traffic off the load queue's FIFO.
                    f = t // GT
                    oap = out_d.ap()
                    for s in SECS:
                        lo, hi = OCH[s]
                        nc.gpsimd.dma_start(
                            out=oap[f * OD + lo:f * OD + hi, :],
                            in_=st[s][:, :])

            # --- software-pipelined main loop (4-deep stage skew) ----------
            x_tiles = {0: load_x(0)}
            live = {}                 # tile -> {"h1"|"h2"|"h3"|"st": ...}
            for g in range(NT + 3):
                if g < NT:
                    t = g
                    f, tl = t // GT, t % GT
                    if tl == 0 and f + 1 < NF:
                        x_tiles[f + 1] = load_x(f + 1)
                    if tl == 0:
                        st = {s: opool.tile([MO[s], FW], BF16,
                                            tag="st" + s, name="st_" + s)
                              for s in SECS}
                    live[t] = {"st": st}
                    live[t]["h1"] = stage_in(t, x_tiles[f])
                if 0 <= g - 1 < NT:
                    lv = live[g - 1]
                    lv["h2"] = stage_hidden(1, lv.pop("h1"))
                if 0 <= g - 2 < NT:
                    lv = live[g - 2]
                    lv["h3"] = stage_hidden(2, lv.pop("h2"))
                if g - 3 >= 0:
                    lv = live.pop(g - 3)
                    stage_out(g - 3, lv.pop("h3"), lv.pop("st"))

    nc.compile()
    return nc


def _prep_inputs(inputs):
    f32 = np.float32

    def wT(name):
        return np.ascontiguousarray(np.asarray(inputs[name], f32).T)

    m = {}
    co_in = np.concatenate([wT("cls_Win"), wT("obj_Win")], 1)   # [64, 128]
    bp_in = np.concatenate([wT("box_Win"), wT("pos_Win")], 1)
    m["sin_co"] = np.concatenate([co_in] * 2, 0)                # dup halves
    m["sin_bp"] = np.concatenate([bp_in] * 2, 0)
    m["sin_ins"] = _bd(wT("ins_Win"), wT("ins_Win"))
    for l in (1, 2):
        m[f"sl{l}_co"] = _bd(np.asarray(inputs["cls_Wh"][l - 1], f32).T,
                             np.asarray(inputs["obj_Wh"][l - 1], f32).T)
        m[f"sl{l}_bp"] = _bd(np.asarray(inputs["box_Wh"][l - 1], f32).T,
                             np.asarray(inputs["pos_Wh"][l - 1], f32).T)
        m[f"sl{l}_ins"] = _bd(np.asarray(inputs["ins_Wh"][l - 1], f32).T,
                              np.asarray(inputs["ins_Wh"][l - 1], f32).T)
    m["sout_co"] = _bd(wT("cls_Wout"), wT("obj_Wout"))          # [128, 83]
    m["sout_bp"] = _bd(wT("box_Wout"), wT("pos_Wout"))          # [128, 68]
    m["sout_ins"] = np.concatenate([wT("ins_Wout")] * 2, 0)     # [128, 128]

    def colv(v):
        return np.asarray(v, f32).reshape(-1)

    bseg = {}
    bseg["bb_in_co"] = np.concatenate([colv(inputs["cls_bin"]),
                                       colv(inputs["obj_bin"])])
    bseg["bb_in_bp"] = np.concatenate([colv(inputs["box_bin"]),
                                       colv(inputs["pos_bin"])])
    bseg["bb_in_ins"] = np.concatenate([colv(inputs["ins_bin"])] * 2)
    for l in (1, 2):
        bseg[f"bb_h{l}_co"] = np.concatenate([colv(inputs["cls_bh"][l - 1]),
                                              colv(inputs["obj_bh"][l - 1])])
        bseg[f"bb_h{l}_bp"] = np.concatenate([colv(inputs["box_bh"][l - 1]),
                                              colv(inputs["pos_bh"][l - 1])])
        bseg[f"bb_h{l}_ins"] = np.concatenate([colv(inputs["ins_bh"][l - 1])] * 2)
    bcol = {}
    bcol["co"] = np.concatenate([colv(inputs["cls_bout"]),
                                 colv(inputs["obj_bout"])])
    bcol["bp"] = np.concatenate([colv(inputs["box_bout"]),
                                 colv(inputs["pos_bout"])])
    bcol["ins"] = colv(inputs["ins_bout"])

    wp = np.zeros((128, WCOLS_TOTAL), f32)
    for name, (c0, n) in _W_COLS.items():
        if name == "ones":
            wp[:, c0:c0 + n] = 1.0
        elif name in m:
            v = m[name]
            wp[:v.shape[0], c0:c0 + n] = v
        else:
            wp[0, c0:c0 + n] = bseg[name]
            wp[64, c0:c0 + n] = bseg[name]
    wp = np.ascontiguousarray(wp)
    import ml_dtypes
    wpf = np.zeros((128, 4), f32)
    wpf[0:MO["co"], 0] = bcol["co"]
    wpf[0:MO["bp"], 1] = bcol["bp"]
    wpf[0:MO["ins"], 2] = bcol["ins"]
    wp = wp.astype(ml_dtypes.bfloat16)

    bin_nz = any(np.any(bseg["bb_in_" + s]) for s in SECS)
    bh_nz = any(np.any(bseg[f"bb_h{l}_" + s]) for s in SECS for l in (1, 2))

    x = np.asarray(inputs["x"], f32)
    in_maps = []
    for c in range(NCORES):
        b, hh = c // 2, c % 2
        xs = x[b, :, hh * 128:(hh + 1) * 128, :].reshape(64, NPX)
        xsr = np.ascontiguousarray(
            np.concatenate([xs[:, :NG], xs[:, NG:]],
                           axis=0)).astype(ml_dtypes.bfloat16)  # [128, NG]
        in_maps.append({"wp": wp, "xs": xsr, "wpf": wpf})
    return in_maps, (bin_nz, bh_nz)


def kernel(**inputs) -> np.ndarray:
    global _last_results
    in_maps, key = _prep_inputs(inputs)
    if key not in _cache:
        _cache[key] = _build(*key)
    nc = _cache[key]
    res = run_bass_kernel_spmd(nc, in_maps, core_ids=list(range(NCORES)))
    _last_results = res

    out = np.empty((B, OD, H, W), np.float32)
    for c in range(NCORES):
        b, hh = c // 2, c % 2
        o = np.asarray(res.results[c]["out"]).astype(np.float32)
        # staged layout: [NF, OD, GT, 2, T] -> per-group pixel-major
        blk = o.reshape(NF, OD, GT, 2, T)
        ga = blk[:, :, :, 0, :].transpose(1, 0, 2, 3).reshape(OD, NG)
        gb = blk[:, :, :, 1, :].transpose(1, 0, 2, 3).reshape(OD, NG)
        core = np.concatenate([ga, gb], axis=1)                 # [OD, NPX]
        out[b, :, hh * 128:(hh + 1) * 128, :] = core.reshape(OD, 128, W)
    return out


# revision 11
# speedup vs baseline: 1.6982x; 1.0788x over previous
"""Trainium2 Bass kernel for the 5-head detection tower (nn_DFD_10849087390476).

Network (per head h of 5): 1x1-conv tower on x [B,64,H,W]:
    h1 = relu(Win x + bin)
    h2 = h1 + relu(Wh0 h1 + bh0)
    h3 = h2 + relu(Wh1 h2 + bh1)
    out_h = Wout h3 + bout
Output = concat over heads: channels [cls 81, obj 2, box 4, pos 64, ins 128] = 279.

Sharding: data-parallel over (batch, H/2) -> 8 shards of 32768 pixels.
Per core the pixels form two 16384-px groups (A, B); a pair-tile is 512 px of
each.  Heads are paired on partition halves: co=(cls top, obj bottom),
bp=(box top, pos bottom), ins=(A top, B bottom).

All GEMMs have K=64.  in-proj and ins-out are 64x128 row-tiled matmul pairs
(tile_position (0,0)/(64,0)) running concurrently on the two PE row halves;
hidden layers and co/bp out are 128x128 block-diagonal.

PSUM is organized as two rings shared by every stage so the PE can run ahead
of evacuation: a 2-bank [128,1024] ring (bufs=3) for co/bp/out tiles and a
1-bank [128,512] ring (bufs=2) for ins tiles -- all 8 banks.  Evacuation:
ACT does the h1 relus and the out-copies (FD=1024, fused per-channel bout
bias); DVE does the residual scalar_tensor_tensor ops.

Output is staged in bf16 and stored with HWDGE (nc.scalar) DMAs into a
contiguous flush-major DRAM layout [NF*279, 4096] (8KB per partition row);
the host de-interleaves and upcasts.  bin/bh biases (zero in the graded net)
ride K=64 accumulating matmuls with the bias vector in row 0 of the
stationary against an all-ones moving operand.
"""
import numpy as np

from concourse import bacc, tile
import concourse.mybir as mybir
from concourse.bass_utils import run_bass_kernel_spmd

F32 = mybir.dt.float32
BF16 = mybir.dt.bfloat16
AF = mybir.ActivationFunctionType
ALU = mybir.AluOpType

B, C, H, W = 4, 64, 256, 256
NCORES = 8
NPX = (B * H * W) // NCORES          # 32768 pixels per core
NG = NPX // 2                        # 16384 per group (A/B)
T = 512                              # pixels per matmul tile
NT = NG // T                         # 32 pair-tiles per core
GT = 4                               # pair-tiles per flush chunk
NF = NT // GT                        # 8 chunks
FW = GT * 2 * T                      # staged columns per flush (4096)
OD = 279                             # output channels

SECS = ("co", "bp", "ins")
MO = {"co": 83, "bp": 68, "ins": 128}
OCH = {"co": (0, 83), "bp": (83, 151), "ins": (151, 279)}

# packed weight-tensor column layout (bf16).  Row-tiled stationaries are
# duplicated on both partition halves; bias-MM segments live in row 0 of
# partitions 0 and 64; out biases are per-partition columns in wpf (f32).
_W_COLS = {}
_c = 0
for _n in ("sin_co", "sin_bp", "sin_ins",
           "sl1_co", "sl1_bp", "sl1_ins", "sl2_co", "sl2_bp", "sl2_ins",
           "sout_ins"):
    _W_COLS[_n] = (_c, 128); _c += 128
_W_COLS["sout_co"] = (_c, MO["co"]); _c += MO["co"]
_W_COLS["sout_bp"] = (_c, MO["bp"]); _c += MO["bp"]
_W_COLS["ones"] = (_c, T); _c += T
for _n in ("bb_in_co", "bb_in_bp", "bb_in_ins",
           "bb_h1_co", "bb_h1_bp", "bb_h1_ins",
           "bb_h2_co", "bb_h2_bp", "bb_h2_ins"):
    _W_COLS[_n] = (_c, 128); _c += 128
WCOLS_TOTAL = _c

_last_results = None                 # test.py reads exec_time_ns from here
_cache = {}


def _bd(a, b):
    out = np.zeros((a.shape[0] + b.shape[0], a.shape[1] + b.shape[1]), np.float32)
    out[:a.shape[0], :a.shape[1]] = a
    out[a.shape[0]:, a.shape[1]:] = b
    return out


def _build(bin_nz: bool, bh_nz: bool):
    nc = bacc.Bacc("TRN2", target_bir_lowering=False, debug=False)

    xs_d = nc.dram_tensor("xs", [128, NG], BF16, kind="ExternalInput")
    wp_d = nc.dram_tensor("wp", [128, WCOLS_TOTAL], BF16, kind="ExternalInput")
    wpf_d = nc.dram_tensor("wpf", [128, 4], F32, kind="ExternalInput")
    out_d = nc.dram_tensor("out", [NF * OD, FW], BF16, kind="ExternalOutput")

    with tile.TileContext(nc) as tc:
        with tc.tile_pool(name="const", bufs=1) as cpool, \
             tc.tile_pool(name="xp", bufs=3) as xpool, \
             tc.tile_pool(name="hp", bufs=2) as hpool, \
             tc.tile_pool(name="op", bufs=3) as opool, \
             tc.tile_pool(name="ps", bufs=1, space="PSUM") as pspool:

            # Only the MM-stationary columns are needed when the bin/bh
            # biases are zero; skip the ones/bias segments in that case.
            used_cols = WCOLS_TOTAL if (bin_nz or bh_nz) else _W_COLS["ones"][0]
            wp_t = cpool.tile([128, used_cols], BF16, tag="wp")
            nc.sync.dma_start(out=wp_t[:], in_=wp_d.ap()[:, 0:used_cols])
            wpf_t = cpool.tile([128, 4], F32, tag="wpf")
            nc.sync.dma_start(out=wpf_t[:], in_=wpf_d.ap())
            # Dummy relu on the tiny bias tile: pulls the ACT table-set load
            # (~2.7us) under the weight DMA instead of the first real relu.
            warm_t = cpool.tile([128, 4], F32, tag="warm")
            nc.scalar.activation(warm_t[:], wpf_t[:], AF.Relu)

            def wap(name, half=None):
                c0, n = _W_COLS[name]
                if half is None:
                    return wp_t[:, c0:c0 + n]
                return wp_t[half * 64:(half + 1) * 64, c0:c0 + n]

            _BCOL_IDX = {"co": 0, "bp": 1, "ins": 2}

            def bap(s):                  # out-bias column AP (f32)
                return wpf_t[0:MO[s], _BCOL_IDX[s]:_BCOL_IDX[s] + 1]

            def mm(out, stat, rhs, start, stop, tp=None):
                nc.tensor.matmul(out, stat, rhs, start=start, stop=stop,
                                 tile_position=tp)

            def bias_mm(p_cols, seg, stop):
                # p_cols[m, n] += seg[m]  (bias in row 0 of a [64,128]
                # stationary, all-ones moving operand, row-tile 0)
                mm(p_cols, wap(seg, 0), wap("ones", 0), False, stop, tp=(0, 0))

            # PSUM rings: tagM holds the 2-bank co/bp tiles (8 allocs per
            # pair-tile vs 3 bufs -- coprime, so slot-reuse WARs always land
            # on an earlier natural dependency, never on the out-copies);
            # tagI holds the 1-bank ins tiles (5 allocs vs 2 bufs).
            def p2(name, rows=128):  # 2-bank [*,1024] psum ring (bufs=3)
                return pspool.tile([rows, 2 * T], F32, tag="p2", name=name,
                                   bufs=3)

            def p1(name):            # 1-bank [128,512] psum ring (bufs=2)
                return pspool.tile([128, T], F32, tag="p1", name=name,
                                   bufs=2)

            def load_x(f):
                x_t = xpool.tile([128, GT * T], BF16, tag="x", name=f"x_{f}")
                nc.sync.dma_start(
                    out=x_t[:],
                    in_=xs_d.ap()[:, f * GT * T:(f + 1) * GT * T])
                return x_t

            # --- stage emitters -------------------------------------------
            # The main loop is software-pipelined 3 deep: emission group g
            # runs in-proj of tile g, hidden-1 of tile g-1, and
            # hidden-2 + out-proj of tile g-2, so every intra-group chain is
            # at most 4 ops and ring WARs land on early/mid evacuations.

            def stage_in(t, x_t):
                xg = x_t[:, (t % GT) * T:(t % GT + 1) * T]
                pco = p2("pco")
                pbp = p2("pbp")
                pin = p1("pin")
                for p, s in ((pco, "co"), (pbp, "bp")):
                    for g in (0, 1):
                        mm(p[:, g * T:(g + 1) * T], wap("sin_" + s, g),
                           xg[g * 64:(g + 1) * 64, :], True, not bin_nz,
                           tp=(g * 64, 0))
                        if bin_nz:
                            bias_mm(p[:, g * T:(g + 1) * T],
                                    "bb_in_" + s, True)
                mm(pin[:], wap("sin_ins"), xg, True, not bin_nz)
                if bin_nz:
                    bias_mm(pin[:], "bb_in_ins", True)

                h1 = {"co": hpool.tile([128, 2 * T], BF16, tag="h1co",
                                       name="h1co"),
                      "bp": hpool.tile([128, 2 * T], BF16, tag="h1bp",
                                       name="h1bp"),
                      "ins": hpool.tile([128, T], BF16, tag="h1ins",
                                        name="h1ins")}
                nc.scalar.activation(h1["co"][:], pco[:], AF.Relu)
                nc.scalar.activation(h1["bp"][:], pbp[:], AF.Relu)
                nc.scalar.activation(h1["ins"][:], pin[:], AF.Relu)
                return h1

            def stage_hidden(l, h):
                lco = p2(f"l{l}co")
                lbp = p2(f"l{l}bp")
                lin = p1(f"l{l}in")
                for p, s in ((lco, "co"), (lbp, "bp")):
                    for g in (0, 1):
                        mm(p[:, g * T:(g + 1) * T], wap(f"sl{l}_" + s),
                           h[s][:, g * T:(g + 1) * T], True, not bh_nz)
                        if bh_nz:
                            bias_mm(p[:, g * T:(g + 1) * T],
                                    f"bb_h{l}_" + s, True)
                mm(lin[:], wap(f"sl{l}_ins"), h["ins"][:], True, not bh_nz)
                if bh_nz:
                    bias_mm(lin[:], f"bb_h{l}_ins", True)
                hn = {"co": hpool.tile([128, 2 * T], BF16, tag=f"h{l + 1}co",
                                       name="hnco"),
                      "bp": hpool.tile([128, 2 * T], BF16, tag=f"h{l + 1}bp",
                                       name="hnbp"),
                      "ins": hpool.tile([128, T], BF16, tag=f"h{l + 1}ins",
                                        name="hnins")}
                for s, p in (("co", lco), ("bp", lbp), ("ins", lin)):
                    nc.vector.scalar_tensor_tensor(
                        hn[s][:], p[:], 0.0, h[s][:], ALU.max, ALU.add)
                return hn

            def stage_out(t, h3, st):
                tl = t % GT
                oco = p2("oco", rows=MO["co"])
                obp = p2("obp", rows=MO["bp"])
                oin = [p1("oin0"), p1("oin1")]
                for g in (0, 1):
                    gc = slice(g * T, (g + 1) * T)
                    mm(oco[:, gc], wap("sout_co"), h3["co"][:, gc],
                       True, True)
                    mm(obp[:, gc], wap("sout_bp"), h3["bp"][:, gc],
                       True, True)
                    mm(oin[g][:], wap("sout_ins", g),
                       h3["ins"][g * 64:(g + 1) * 64, :], True, True,
                       tp=(g * 64, 0))
                col = slice(tl * 2 * T, (tl + 1) * 2 * T)
                for s, p in (("co", oco), ("bp", obp)):
                    nc.scalar.activation(st[s][:, col], p[:],
                                         AF.Identity, bias=bap(s),
                                         scale=1.0)
                for g in (0, 1):
                    gcol = slice(tl * 2 * T + g * T,
                                 tl * 2 * T + (g + 1) * T)
                    nc.scalar.activation(st["ins"][:, gcol], oin[g][:],
                                         AF.Identity, bias=bap("ins"),
                                         scale=1.0)
                if tl in (GT // 2 - 1, GT - 1):
                    # SWDGE stores: gpsimd is otherwise idle, its descriptor
                    # swizzle spreads evenly over all 16 SDMA engines, and it
                    # keeps the store traffic off the load queue's FIFO.
                    # Half-flush granularity halves the post-compute drain.
                    f = t // GT
                    half = 0 if tl == GT // 2 - 1 else 1
                    hw = FW // 2
                    cs = slice(half * hw, (half + 1) * hw)
                    oap = out_d.ap()
                    for s in SECS:
                        lo, hi = OCH[s]
                        nc.gpsimd.dma_start(
                            out=oap[f * OD + lo:f * OD + hi, cs],
                            in_=st[s][:, cs])

            # --- software-pipelined main loop (4-deep stage skew) ----------
            x_tiles = {0: load_x(0)}
            live = {}                 # tile -> {"h1"|"h2"|"h3"|"st": ...}
            for g in range(NT + 3):
                if g < NT:
                    t = g
                    f, tl = t // GT, t % GT
                    if tl == 0 and f + 1 < NF:
                        x_tiles[f + 1] = load_x(f + 1)
                    if tl == 0:
                        st = {s: opool.tile([MO[s], FW], BF16,
                                            tag="st" + s, name="st_" + s)
                              for s in SECS}
                    live[t] = {"st": st}
                    live[t]["h1"] = stage_in(t, x_tiles[f])
                if 0 <= g - 1 < NT:
                    lv = live[g - 1]
                    lv["h2"] = stage_hidden(1, lv.pop("h1"))
                if 0 <= g - 2 < NT:
                    lv = live[g - 2]
                    lv["h3"] = stage_hidden(2, lv.pop("h2"))
                if g - 3 >= 0:
                    lv = live.pop(g - 3)
                    stage_out(g - 3, lv.pop("h3"), lv.pop("st"))

    nc.compile()
    return nc


def _prep_inputs(inputs):
    f32 = np.float32

    def wT(name):
        return np.ascontiguousarray(np.asarray(inputs[name], f32).T)

    m = {}
    co_in = np.concatenate([wT("cls_Win"), wT("obj_Win")], 1)   # [64, 128]
    bp_in = np.concatenate([wT("box_Win"), wT("pos_Win")], 1)
    m["sin_co"] = np.concatenate([co_in] * 2, 0)                # dup halves
    m["sin_bp"] = np.concatenate([bp_in] * 2, 0)
    m["sin_ins"] = _bd(wT("ins_Win"), wT("ins_Win"))
    for l in (1, 2):
        m[f"sl{l}_co"] = _bd(np.asarray(inputs["cls_Wh"][l - 1], f32).T,
                             np.asarray(inputs["obj_Wh"][l - 1], f32).T)
        m[f"sl{l}_bp"] = _bd(np.asarray(inputs["box_Wh"][l - 1], f32).T,
                             np.asarray(inputs["pos_Wh"][l - 1], f32).T)
        m[f"sl{l}_ins"] = _bd(np.asarray(inputs["ins_Wh"][l - 1], f32).T,
                              np.asarray(inputs["ins_Wh"][l - 1], f32).T)
    m["sout_co"] = _bd(wT("cls_Wout"), wT("obj_Wout"))          # [128, 83]
    m["sout_bp"] = _bd(wT("box_Wout"), wT("pos_Wout"))          # [128, 68]
    m["sout_ins"] = np.concatenate([wT("ins_Wout")] * 2, 0)     # [128, 128]

    def colv(v):
        return np.asarray(v, f32).reshape(-1)

    bseg = {}
    bseg["bb_in_co"] = np.concatenate([colv(inputs["cls_bin"]),
                                       colv(inputs["obj_bin"])])
    bseg["bb_in_bp"] = np.concatenate([colv(inputs["box_bin"]),
                                       colv(inputs["pos_bin"])])
    bseg["bb_in_ins"] = np.concatenate([colv(inputs["ins_bin"])] * 2)
    for l in (1, 2):
        bseg[f"bb_h{l}_co"] = np.concatenate([colv(inputs["cls_bh"][l - 1]),
                                              colv(inputs["obj_bh"][l - 1])])
        bseg[f"bb_h{l}_bp"] = np.concatenate([colv(inputs["box_bh"][l - 1]),
                                              colv(inputs["pos_bh"][l - 1])])
        bseg[f"bb_h{l}_ins"] = np.concatenate([colv(inputs["ins_bh"][l - 1])] * 2)
    bcol = {}
    bcol["co"] = np.concatenate([colv(inputs["cls_bout"]),
                                 colv(inputs["obj_bout"])])
    bcol["bp"] = np.concatenate([colv(inputs["box_bout"]),
                                 colv(inputs["pos_bout"])])
    bcol["ins"] = colv(inputs["ins_bout"])

    wp = np.zeros((128, WCOLS_TOTAL), f32)
    for name, (c0, n) in _W_COLS.items():
        if name == "ones":
            wp[:, c0:c0 + n] = 1.0
        elif name in m:
            v = m[name]
            wp[:v.shape[0], c0:c0 + n] = v
        else:
            wp[0, c0:c0 + n] = bseg[name]
            wp[64, c0:c0 + n] = bseg[name]
    wp = np.ascontiguousarray(wp)
    import ml_dtypes
    wpf = np.zeros((128, 4), f32)
    wpf[0:MO["co"], 0] = bcol["co"]
    wpf[0:MO["bp"], 1] = bcol["bp"]
    wpf[0:MO["ins"], 2] = bcol["ins"]
    wp = wp.astype(ml_dtypes.bfloat16)

    bin_nz = any(np.any(bseg["bb_in_" + s]) for s in SECS)
    bh_nz = any(np.any(bseg[f"bb_h{l}_" + s]) for s in SECS for l in (1, 2))

    x = np.asarray(inputs["x"], f32)
    in_maps = []
    for c in range(NCORES):
        b, hh = c // 2, c % 2
        xs = x[b, :, hh * 128:(hh + 1) * 128, :].reshape(64, NPX)
        xsr = np.ascontiguousarray(
            np.concatenate([xs[:, :NG], xs[:, NG:]],
                           axis=0)).astype(ml_dtypes.bfloat16)  # [128, NG]
        in_maps.append({"wp": wp, "xs": xsr, "wpf": wpf})
    return in_maps, (bin_nz, bh_nz)


def kernel(**inputs) -> np.ndarray:
    global _last_results
    in_maps, key = _prep_inputs(inputs)
    if key not in _cache:
        _cache[key] = _build(*key)
    nc = _cache[key]
    res = run_bass_kernel_spmd(nc, in_maps, core_ids=list(range(NCORES)))
    _last_results = res

    out = np.empty((B, OD, H, W), np.float32)
    for c in range(NCORES):
        b, hh = c // 2, c % 2
        o = np.asarray(res.results[c]["out"]).astype(np.float32)
        # staged layout: [NF, OD, GT, 2, T] -> per-group pixel-major
        blk = o.reshape(NF, OD, GT, 2, T)
        ga = blk[:, :, :, 0, :].transpose(1, 0, 2, 3).reshape(OD, NG)
        gb = blk[:, :, :, 1, :].transpose(1, 0, 2, 3).reshape(OD, NG)
        core = np.concatenate([ga, gb], axis=1)                 # [OD, NPX]
        out[b, :, hh * 128:(hh + 1) * 128, :] = core.reshape(OD, 128, W)
    return out


# revision 14
# speedup vs baseline: 1.7107x; 1.0074x over previous
"""Trainium2 Bass kernel for the 5-head detection tower (nn_DFD_10849087390476).

Network (per head h of 5): 1x1-conv tower on x [B,64,H,W]:
    h1 = relu(Win x + bin)
    h2 = h1 + relu(Wh0 h1 + bh0)
    h3 = h2 + relu(Wh1 h2 + bh1)
    out_h = Wout h3 + bout
Output = concat over heads: channels [cls 81, obj 2, box 4, pos 64, ins 128] = 279.

Sharding: data-parallel over (batch, H/2) -> 8 shards of 32768 pixels.
Per core the pixels form two 16384-px groups (A, B); a pair-tile is 512 px of
each.  Heads are paired on partition halves: co=(cls top, obj bottom),
bp=(box top, pos bottom), ins=(A top, B bottom).

All GEMMs have K=64.  in-proj and ins-out are 64x128 row-tiled matmul pairs
(tile_position (0,0)/(64,0)) running concurrently on the two PE row halves;
hidden layers and co/bp out are 128x128 block-diagonal.

PSUM is organized as two rings shared by every stage so the PE can run ahead
of evacuation: a 2-bank [128,1024] ring (bufs=3) for co/bp/out tiles and a
1-bank [128,512] ring (bufs=2) for ins tiles -- all 8 banks.  Evacuation:
ACT does the h1 relus and the out-copies (FD=1024, fused per-channel bout
bias); DVE does the residual scalar_tensor_tensor ops.

Output is staged in bf16 and stored with HWDGE (nc.scalar) DMAs into a
contiguous flush-major DRAM layout [NF*279, 4096] (8KB per partition row);
the host de-interleaves and upcasts.  bin/bh biases (zero in the graded net)
ride K=64 accumulating matmuls with the bias vector in row 0 of the
stationary against an all-ones moving operand.
"""
import numpy as np

from concourse import bacc, tile
import concourse.mybir as mybir
from concourse.bass_utils import run_bass_kernel_spmd

F32 = mybir.dt.float32
BF16 = mybir.dt.bfloat16
AF = mybir.ActivationFunctionType
ALU = mybir.AluOpType

B, C, H, W = 4, 64, 256, 256
NCORES = 8
NPX = (B * H * W) // NCORES          # 32768 pixels per core
NG = NPX // 2                        # 16384 per group (A/B)
T = 512                              # pixels per matmul tile
NT = NG // T                         # 32 pair-tiles per core
GT = 4                               # pair-tiles per flush chunk
NF = NT // GT                        # 8 chunks
FW = GT * 2 * T                      # staged columns per flush (4096)
OD = 279                             # output channels

SECS = ("co", "bp", "ins")
MO = {"co": 83, "bp": 68, "ins": 128}
OCH = {"co": (0, 83), "bp": (83, 151), "ins": (151, 279)}

# packed weight-tensor column layout (bf16).  Row-tiled stationaries are
# duplicated on both partition halves; bias-MM segments live in row 0 of
# partitions 0 and 64; out biases are per-partition columns in wpf (f32).
_W_COLS = {}
_c = 0
for _n in ("sin_co", "sin_bp", "sin_ins",
           "sl1_co", "sl1_bp", "sl1_ins", "sl2_co", "sl2_bp", "sl2_ins",
           "sout_ins"):
    _W_COLS[_n] = (_c, 128); _c += 128
_W_COLS["sout_co"] = (_c, MO["co"]); _c += MO["co"]
_W_COLS["sout_bp"] = (_c, MO["bp"]); _c += MO["bp"]
_W_COLS["ones"] = (_c, T); _c += T
for _n in ("bb_in_co", "bb_in_bp", "bb_in_ins",
           "bb_h1_co", "bb_h1_bp", "bb_h1_ins",
           "bb_h2_co", "bb_h2_bp", "bb_h2_ins"):
    _W_COLS[_n] = (_c, 128); _c += 128
WCOLS_TOTAL = _c

_last_results = None                 # test.py reads exec_time_ns from here
_cache = {}


def _bd(a, b):
    out = np.zeros((a.shape[0] + b.shape[0], a.shape[1] + b.shape[1]), np.float32)
    out[:a.shape[0], :a.shape[1]] = a
    out[a.shape[0]:, a.shape[1]:] = b
    return out


def _build(bin_nz: bool, bh_nz: bool):
    nc = bacc.Bacc("TRN2", target_bir_lowering=False, debug=False)

    xs_d = nc.dram_tensor("xs", [128, NG], BF16, kind="ExternalInput")
    wp_d = nc.dram_tensor("wp", [128, WCOLS_TOTAL], BF16, kind="ExternalInput")
    wpf_d = nc.dram_tensor("wpf", [128, 4], F32, kind="ExternalInput")
    out_d = nc.dram_tensor("out", [NF * OD, FW], BF16, kind="ExternalOutput")

    with tile.TileContext(nc) as tc:
        with tc.tile_pool(name="const", bufs=1) as cpool, \
             tc.tile_pool(name="xp", bufs=3) as xpool, \
             tc.tile_pool(name="hp", bufs=2) as hpool, \
             tc.tile_pool(name="op", bufs=3) as opool, \
             tc.tile_pool(name="ps", bufs=1, space="PSUM") as pspool:

            # Only the MM-stationary columns are needed when the bin/bh
            # biases are zero; skip the ones/bias segments in that case.
            used_cols = WCOLS_TOTAL if (bin_nz or bh_nz) else _W_COLS["ones"][0]
            wpf_t = cpool.tile([128, 4], F32, tag="wpf")
            nc.sync.dma_start(out=wpf_t[:], in_=wpf_d.ap())
            wp_t = cpool.tile([128, used_cols], BF16, tag="wp")
            nc.sync.dma_start(out=wp_t[:], in_=wp_d.ap()[:, 0:used_cols])
            # Dummy relu on the tiny bias tile: pulls the ACT table-set load
            # (~2.7us) under the weight DMA instead of the first real relu.
            warm_t = cpool.tile([128, 4], F32, tag="warm")
            nc.scalar.activation(warm_t[:], wpf_t[:], AF.Relu)

            def wap(name, half=None):
                c0, n = _W_COLS[name]
                if half is None:
                    return wp_t[:, c0:c0 + n]
                return wp_t[half * 64:(half + 1) * 64, c0:c0 + n]

            _BCOL_IDX = {"co": 0, "bp": 1, "ins": 2}

            def bap(s):                  # out-bias column AP (f32)
                return wpf_t[0:MO[s], _BCOL_IDX[s]:_BCOL_IDX[s] + 1]

            def mm(out, stat, rhs, start, stop, tp=None):
                nc.tensor.matmul(out, stat, rhs, start=start, stop=stop,
                                 tile_position=tp)

            def bias_mm(p_cols, seg, stop):
                # p_cols[m, n] += seg[m]  (bias in row 0 of a [64,128]
                # stationary, all-ones moving operand, row-tile 0)
                mm(p_cols, wap(seg, 0), wap("ones", 0), False, stop, tp=(0, 0))

            # PSUM rings: tagM holds the 2-bank co/bp tiles (8 allocs per
            # pair-tile vs 3 bufs -- coprime, so slot-reuse WARs always land
            # on an earlier natural dependency, never on the out-copies);
            # tagI holds the 1-bank ins tiles (5 allocs vs 2 bufs).
            def p2(name, rows=128):  # 2-bank [*,1024] psum ring (bufs=3)
                return pspool.tile([rows, 2 * T], F32, tag="p2", name=name,
                                   bufs=3)

            def p1(name):            # 1-bank [128,512] psum ring (bufs=2)
                return pspool.tile([128, T], F32, tag="p1", name=name,
                                   bufs=2)

            def load_x(f):
                x_t = xpool.tile([128, GT * T], BF16, tag="x", name=f"x_{f}")
                # chunk 0 rides the scalar HWDGE queue so it overlaps the
                # weight load on the sync queue (ACT is idle at kernel start)
                eng = nc.scalar if f == 0 else nc.sync
                eng.dma_start(
                    out=x_t[:],
                    in_=xs_d.ap()[:, f * GT * T:(f + 1) * GT * T])
                return x_t

            # --- stage emitters -------------------------------------------
            # The main loop is software-pipelined 3 deep: emission group g
            # runs in-proj of tile g, hidden-1 of tile g-1, and
            # hidden-2 + out-proj of tile g-2, so every intra-group chain is
            # at most 4 ops and ring WARs land on early/mid evacuations.

            def stage_in(t, x_t):
                xg = x_t[:, (t % GT) * T:(t % GT + 1) * T]
                pco = p2("pco")
                pbp = p2("pbp")
                pin = p1("pin")
                for p, s in ((pco, "co"), (pbp, "bp")):
                    for g in (0, 1):
                        mm(p[:, g * T:(g + 1) * T], wap("sin_" + s, g),
                           xg[g * 64:(g + 1) * 64, :], True, not bin_nz,
                           tp=(g * 64, 0))
                        if bin_nz:
                            bias_mm(p[:, g * T:(g + 1) * T],
                                    "bb_in_" + s, True)
                mm(pin[:], wap("sin_ins"), xg, True, not bin_nz)
                if bin_nz:
                    bias_mm(pin[:], "bb_in_ins", True)

                h1 = {"co": hpool.tile([128, 2 * T], BF16, tag="h1co",
                                       name="h1co"),
                      "bp": hpool.tile([128, 2 * T], BF16, tag="h1bp",
                                       name="h1bp"),
                      "ins": hpool.tile([128, T], BF16, tag="h1ins",
                                        name="h1ins")}
                nc.scalar.activation(h1["co"][:], pco[:], AF.Relu)
                nc.scalar.activation(h1["bp"][:], pbp[:], AF.Relu)
                nc.scalar.activation(h1["ins"][:], pin[:], AF.Relu)
                return h1

            def stage_hidden(l, h):
                lco = p2(f"l{l}co")
                lbp = p2(f"l{l}bp")
                lin = p1(f"l{l}in")
                for p, s in ((lco, "co"), (lbp, "bp")):
                    for g in (0, 1):
                        mm(p[:, g * T:(g + 1) * T], wap(f"sl{l}_" + s),
                           h[s][:, g * T:(g + 1) * T], True, not bh_nz)
                        if bh_nz:
                            bias_mm(p[:, g * T:(g + 1) * T],
                                    f"bb_h{l}_" + s, True)
                mm(lin[:], wap(f"sl{l}_ins"), h["ins"][:], True, not bh_nz)
                if bh_nz:
                    bias_mm(lin[:], f"bb_h{l}_ins", True)
                hn = {"co": hpool.tile([128, 2 * T], BF16, tag=f"h{l + 1}co",
                                       name="hnco"),
                      "bp": hpool.tile([128, 2 * T], BF16, tag=f"h{l + 1}bp",
                                       name="hnbp"),
                      "ins": hpool.tile([128, T], BF16, tag=f"h{l + 1}ins",
                                        name="hnins")}
                for s, p in (("co", lco), ("bp", lbp), ("ins", lin)):
                    nc.vector.scalar_tensor_tensor(
                        hn[s][:], p[:], 0.0, h[s][:], ALU.max, ALU.add)
                return hn

            def stage_out(t, h3, st):
                tl = t % GT
                oco = p2("oco", rows=MO["co"])
                obp = p2("obp", rows=MO["bp"])
                oin = [p1("oin0"), p1("oin1")]
                for g in (0, 1):
                    gc = slice(g * T, (g + 1) * T)
                    mm(oco[:, gc], wap("sout_co"), h3["co"][:, gc],
                       True, True)
                    mm(obp[:, gc], wap("sout_bp"), h3["bp"][:, gc],
                       True, True)
                    mm(oin[g][:], wap("sout_ins", g),
                       h3["ins"][g * 64:(g + 1) * 64, :], True, True,
                       tp=(g * 64, 0))
                col = slice(tl * 2 * T, (tl + 1) * 2 * T)
                for s, p in (("co", oco), ("bp", obp)):
                    nc.scalar.activation(st[s][:, col], p[:],
                                         AF.Identity, bias=bap(s),
                                         scale=1.0)
                for g in (0, 1):
                    gcol = slice(tl * 2 * T + g * T,
                                 tl * 2 * T + (g + 1) * T)
                    nc.scalar.activation(st["ins"][:, gcol], oin[g][:],
                                         AF.Identity, bias=bap("ins"),
                                         scale=1.0)
                # SWDGE stores: gpsimd is otherwise idle, its descriptor
                # swizzle spreads evenly over all 16 SDMA engines, and it
                # keeps the store traffic off the load queue's FIFO.
                # Half-flush granularity bounds the post-compute drain; the
                # last flush stores per-tile so the final drain is minimal.
                f = t // GT
                last = f == NF - 1
                if last or tl in (GT // 2 - 1, GT - 1):
                    if last:
                        cs = slice(tl * 2 * T, (tl + 1) * 2 * T)
                    else:
                        half = 0 if tl == GT // 2 - 1 else 1
                        cs = slice(half * (FW // 2), (half + 1) * (FW // 2))
                    oap = out_d.ap()
                    for s in SECS:
                        lo, hi = OCH[s]
                        nc.gpsimd.dma_start(
                            out=oap[f * OD + lo:f * OD + hi, cs],
                            in_=st[s][:, cs])

            # --- software-pipelined main loop (4-deep stage skew) ----------
            x_tiles = {0: load_x(0)}
            live = {}                 # tile -> {"h1"|"h2"|"h3"|"st": ...}
            for g in range(NT + 3):
                if g < NT:
                    t = g
                    f, tl = t // GT, t % GT
                    if tl == 0 and f + 1 < NF:
                        x_tiles[f + 1] = load_x(f + 1)
                    if tl == 0:
                        st = {s: opool.tile([MO[s], FW], BF16,
                                            tag="st" + s, name="st_" + s)
                              for s in SECS}
                    live[t] = {"st": st}
                    live[t]["h1"] = stage_in(t, x_tiles[f])
                if 0 <= g - 1 < NT:
                    lv = live[g - 1]
                    lv["h2"] = stage_hidden(1, lv.pop("h1"))
                if 0 <= g - 2 < NT:
                    lv = live[g - 2]
                    lv["h3"] = stage_hidden(2, lv.pop("h2"))
                if g - 3 >= 0:
                    lv = live.pop(g - 3)
                    stage_out(g - 3, lv.pop("h3"), lv.pop("st"))

    nc.compile()
    return nc


def _prep_inputs(inputs):
    f32 = np.float32

    def wT(name):
        return np.ascontiguousarray(np.asarray(inputs[name], f32).T)

    m = {}
    co_in = np.concatenate([wT("cls_Win"), wT("obj_Win")], 1)   # [64, 128]
    bp_in = np.concatenate([wT("box_Win"), wT("pos_Win")], 1)
    m["sin_co"] = np.concatenate([co_in] * 2, 0)                # dup halves
    m["sin_bp"] = np.concatenate([bp_in] * 2, 0)
    m["sin_ins"] = _bd(wT("ins_Win"), wT("ins_Win"))
    for l in (1, 2):
        m[f"sl{l}_co"] = _bd(np.asarray(inputs["cls_Wh"][l - 1], f32).T,
                             np.asarray(inputs["obj_Wh"][l - 1], f32).T)
        m[f"sl{l}_bp"] = _bd(np.asarray(inputs["box_Wh"][l - 1], f32).T,
                             np.asarray(inputs["pos_Wh"][l - 1], f32).T)
        m[f"sl{l}_ins"] = _bd(np.asarray(inputs["ins_Wh"][l - 1], f32).T,
                              np.asarray(inputs["ins_Wh"][l - 1], f32).T)
    m["sout_co"] = _bd(wT("cls_Wout"), wT("obj_Wout"))          # [128, 83]
    m["sout_bp"] = _bd(wT("box_Wout"), wT("pos_Wout"))          # [128, 68]
    m["sout_ins"] = np.concatenate([wT("ins_Wout")] * 2, 0)     # [128, 128]

    def colv(v):
        return np.asarray(v, f32).reshape(-1)

    bseg = {}
    bseg["bb_in_co"] = np.concatenate([colv(inputs["cls_bin"]),
                                       colv(inputs["obj_bin"])])
    bseg["bb_in_bp"] = np.concatenate([colv(inputs["box_bin"]),
                                       colv(inputs["pos_bin"])])
    bseg["bb_in_ins"] = np.concatenate([colv(inputs["ins_bin"])] * 2)
    for l in (1, 2):
        bseg[f"bb_h{l}_co"] = np.concatenate([colv(inputs["cls_bh"][l - 1]),
                                              colv(inputs["obj_bh"][l - 1])])
        bseg[f"bb_h{l}_bp"] = np.concatenate([colv(inputs["box_bh"][l - 1]),
                                              colv(inputs["pos_bh"][l - 1])])
        bseg[f"bb_h{l}_ins"] = np.concatenate([colv(inputs["ins_bh"][l - 1])] * 2)
    bcol = {}
    bcol["co"] = np.concatenate([colv(inputs["cls_bout"]),
                                 colv(inputs["obj_bout"])])
    bcol["bp"] = np.concatenate([colv(inputs["box_bout"]),
                                 colv(inputs["pos_bout"])])
    bcol["ins"] = colv(inputs["ins_bout"])

    wp = np.zeros((128, WCOLS_TOTAL), f32)
    for name, (c0, n) in _W_COLS.items():
        if name == "ones":
            wp[:, c0:c0 + n] = 1.0
        elif name in m:
            v = m[name]
            wp[:v.shape[0], c0:c0 + n] = v
        else:
            wp[0, c0:c0 + n] = bseg[name]
            wp[64, c0:c0 + n] = bseg[name]
    wp = np.ascontiguousarray(wp)
    import ml_dtypes
    wpf = np.zeros((128, 4), f32)
    wpf[0:MO["co"], 0] = bcol["co"]
    wpf[0:MO["bp"], 1] = bcol["bp"]
    wpf[0:MO["ins"], 2] = bcol["ins"]
    wp = wp.astype(ml_dtypes.bfloat16)

    bin_nz = any(np.any(bseg["bb_in_" + s]) for s in SECS)
    bh_nz = any(np.any(bseg[f"bb_h{l}_" + s]) for s in SECS for l in (1, 2))

    x = np.asarray(inputs["x"], f32)
    in_maps = []
    for c in range(NCORES):
        b, hh = c // 2, c % 2
        xs = x[b, :, hh * 128:(hh + 1) * 128, :].reshape(64, NPX)
        xsr = np.ascontiguousarray(
            np.concatenate([xs[:, :NG], xs[:, NG:]],
                           axis=0)).astype(ml_dtypes.bfloat16)  # [128, NG]
        in_maps.append({"wp": wp, "xs": xsr, "wpf": wpf})
    return in_maps, (bin_nz, bh_nz)


def kernel(**inputs) -> np.ndarray:
    global _last_results
    in_maps, key = _prep_inputs(inputs)
    if key not in _cache:
        _cache[key] = _build(*key)
    nc = _cache[key]
    res = run_bass_kernel_spmd(nc, in_maps, core_ids=list(range(NCORES)))
    _last_results = res

    out = np.empty((B, OD, H, W), np.float32)
    for c in range(NCORES):
        b, hh = c // 2, c % 2
        o = np.asarray(res.results[c]["out"]).astype(np.float32)
        # staged layout: [NF, OD, GT, 2, T] -> per-group pixel-major
        blk = o.reshape(NF, OD, GT, 2, T)
        ga = blk[:, :, :, 0, :].transpose(1, 0, 2, 3).reshape(OD, NG)
        gb = blk[:, :, :, 1, :].transpose(1, 0, 2, 3).reshape(OD, NG)
        core = np.concatenate([ga, gb], axis=1)                 # [OD, NPX]
        out[b, :, hh * 128:(hh + 1) * 128, :] = core.reshape(OD, 128, W)
    return out
